# revision 10
# baseline (speedup 1.0000x reference)
"""Trainium2 Bass kernel for nn_Net_13486197310235 (GNN message passing).

Data-parallel over graphs: 8 cores x 32 graphs each. All MLP compute is done
as feature-major matmuls on the PE array in bf16 (fp32 PSUM accumulation).
Per-edge gathers x[row]/x[col] use the factored form (project nodes once,
then expand with per-graph-pair one-hot matmuls); scatter-mean uses one-hot
matmuls with 1/deg folded into the selection matrix. BatchNorm statistics are
computed on-device with a cross-core AllReduce.

Self-contained: hardcodes the problem shapes (B=256 graphs, 64 nodes/graph,
512 edges/graph, NF=32, EF=16).
"""

import numpy as np
import ml_dtypes

import concourse.bacc as bacc
import concourse.bass as bass
import concourse.mybir as mybir
import concourse.tile as tile
from concourse.bass_utils import run_bass_kernel_spmd
from concourse.masks import make_identity

NCORES = 8
B, NPG, EPG = 256, 64, 512
NF, EF = 32, 16
N, E = B * NPG, B * EPG

GPC = B // NCORES          # graphs per core = 32
NPC = GPC * NPG            # nodes per core = 2048
EPC = GPC * EPG            # edges per core = 16384
PAIRS = GPC // 2           # graph pairs per core = 16
CHUNK = 512                # edges per compute chunk
NCH = EPC // CHUNK         # chunks per core = 32
SEL_PG = 2                 # selected edges per graph
SELC = GPC * SEL_PG        # selected edges per core = 64

HDT = mybir.dt.float16
F32 = mybir.dt.float32
nph = np.float16

ALU = mybir.AluOpType
ACTF = mybir.ActivationFunctionType

_CACHE = {}


# ---------------------------------------------------------------- host pack

class Pack:
    """Accumulates weight blocks ([128, w] K-blocks) and bias blocks."""

    def __init__(self):
        self.w = {128: [], 256: [], 512: []}
        self.bias = []
        self.idx = {}       # key -> (width, start, nblocks)
        self.bidx = {}      # key -> (start, nblocks)
        self.layer_w = {}   # (layer, width) -> [start, count]

    def mark_layer(self, layer):
        self._layer = layer
        for w in (128, 256, 512):
            self.layer_w[(layer, w)] = [len(self.w[w]), 0]

    def add_w(self, key, W, width):
        W = np.asarray(W, np.float32)
        din, dout = W.shape
        assert dout <= width
        kb = -(-din // 128)
        Wp = np.zeros((kb * 128, width), np.float32)
        Wp[:din, :dout] = W
        start = len(self.w[width])
        for k in range(kb):
            self.w[width].append(Wp[k * 128:(k + 1) * 128])
        self.idx[key] = (width, start, kb)
        self.layer_w[(self._layer, width)][1] += kb

    def add_b(self, key, b):
        b = np.asarray(b, np.float32).reshape(-1)
        nb = -(-len(b) // 128)
        bp = np.zeros(nb * 128, np.float32)
        bp[:len(b)] = b
        start = len(self.bias)
        for k in range(nb):
            self.bias.append(bp[k * 128:(k + 1) * 128])
        self.bidx[key] = (start, nb)


def pack_weights(params):
    pk = Pack()
    for l in range(1, 7):
        pk.mark_layer(l)
        # edge MLP
        (W1, b1), (W2, b2), (W3, b3) = params[f'e{l}']
        nfx = NF if l == 1 else 128
        W1 = np.asarray(W1, np.float32)
        pk.add_w(f'e{l}r', W1[:nfx], 128)
        pk.add_w(f'e{l}c', W1[nfx:2 * nfx], 128)
        pk.add_w(f'e{l}e', W1[2 * nfx:], 128)
        pk.add_w(f'e{l}2', W2, 128)
        pk.add_w(f'e{l}3', W3, 512 if l < 6 else 128)
        pk.add_b(f'e{l}1', b1)
        pk.add_b(f'e{l}2', b2)
        pk.add_b(f'e{l}3', b3)
        if l == 6:
            break
        # node MLP a
        (Wa1, ba1), (Wa2, ba2), (Wa3, ba3) = params[f'n{l}a']
        Wa1 = np.asarray(Wa1, np.float32)
        pk.add_w(f'na{l}x', Wa1[:nfx], 256)
        pk.add_w(f'na{l}e', Wa1[nfx:], 256)
        pk.add_w(f'na{l}2', Wa2, 256)
        pk.add_w(f'na{l}3', Wa3, 256)
        pk.add_b(f'na{l}1', ba1)
        pk.add_b(f'na{l}2', ba2)
        # ba3 handled via replicated tile
        # node MLP b
        (Wb1, bb1), (Wb2, bb2) = params[f'n{l}b']
        Wb1 = np.asarray(Wb1, np.float32)
        pk.add_w(f'nb{l}x', Wb1[:nfx], 256)
        pk.add_w(f'nb{l}a', Wb1[nfx:], 256)
        pk.add_w(f'nb{l}2', Wb2, 128)
        pk.add_b(f'nb{l}1', bb1)
        pk.add_b(f'nb{l}2', bb2)
    pk.mark_layer(7)
    (Wh1, bh1), (Wh2, bh2) = params['head']
    pk.add_w('h1', Wh1, 128)
    pk.add_w('h2', Wh2, 128)
    pk.add_b('h1', bh1)
    pk.add_b('h2', bh2)

    ba3rep = np.zeros((128, 5, 256), np.float32)
    for l in range(1, 6):
        ba3rep[:, l - 1, :] = np.asarray(params[f'n{l}a'][2][1], np.float32)[None, :]
    return pk, ba3rep


# ---------------------------------------------------------------- host shard

def prep_inputs(inputs):
    """Shard + reorder + build one-hot matrices. Returns per-core input maps
    plus the pack object (weights are replicated)."""
    params = inputs['params']
    x = np.asarray(inputs['x'], np.float32)
    ea = np.asarray(inputs['edge_attr'], np.float32)
    ei = np.asarray(inputs['edge_index'], np.int64)
    batch = np.asarray(inputs['batch'], np.int64)
    sel_idx = np.asarray(inputs['sel_idx'], np.int64)
    eb = np.asarray(inputs['eb'], np.int64)

    row, col = ei[0], ei[1]
    gid = batch[row]
    assert np.array_equal(batch[col], gid), "edges must stay within one graph"
    # group edges by graph (stable) -> per-graph contiguous blocks of EPG
    perm = np.argsort(gid, kind='stable')
    counts = np.bincount(gid, minlength=B)
    assert np.all(counts == EPG), "expected uniform edges/graph"
    assert np.array_equal(batch, np.repeat(np.arange(B), NPG)), \
        "expected uniform nodes/graph"
    row_s, col_s, ea_s = row[perm], col[perm], ea[perm]
    inv = np.empty(E, np.int64)
    inv[perm] = np.arange(E)
    sel_pos = inv[sel_idx]            # position in reordered edge list
    # selected edges sorted by graph
    sel_g = eb
    assert np.all(np.bincount(sel_g, minlength=B) == SEL_PG)
    sord = np.argsort(sel_g, kind='stable')
    sel_pos = sel_pos[sord]
    sel_g = sel_g[sord]

    # in-degree over col (per node), computed per graph block
    deg = np.bincount(col_s, minlength=N).astype(np.float32)
    invdeg = 1.0 / np.maximum(deg, 1.0)

    pk, ba3rep = pack_weights(params)
    w128 = np.stack(pk.w[128]).astype(nph)
    w256 = np.stack(pk.w[256]).astype(nph)
    w512 = np.stack(pk.w[512]).astype(nph)
    biasT = np.stack(pk.bias, axis=1).astype(np.float32)     # [128, NB]
    ba3rep_h = ba3rep.reshape(128, 5 * 256)

    bn_g = np.asarray(params['bn_node'][0], np.float32)[None, :]
    bn_b = np.asarray(params['bn_node'][1], np.float32)[None, :]
    be_g = np.asarray(params['bn_edge'][0], np.float32)[None, :]
    be_b = np.asarray(params['bn_edge'][1], np.float32)[None, :]

    in_maps = []
    sel_copy_meta = None
    for cidx in range(NCORES):
        g0 = cidx * GPC
        n0, e0 = g0 * NPG, g0 * EPG
        xs = x[n0:n0 + NPC]                       # [2048, 32]
        eas = ea_s[e0:e0 + EPC]                   # [16384, 16]
        rows = row_s[e0:e0 + EPC] - n0            # local node idx 0..2047
        cols = col_s[e0:e0 + EPC] - n0

        # em layouts with partition dim = row-within-128-tile
        x_em = xs.reshape(NPC // 128, 128, NF).transpose(1, 0, 2).reshape(128, -1)
        e_em = eas.reshape(EPC // 128, 128, EF).transpose(1, 0, 2).reshape(128, -1)

        eidx = np.arange(EPC)
        pair = eidx // (2 * EPG)                  # pair of each edge
        rloc = rows - pair * 128                  # node idx within pair: 0..127
        cloc = cols - pair * 128
        assert rloc.min() >= 0 and rloc.max() < 128
        grow = np.zeros((128, EPC), np.float32)
        gcol = np.zeros((128, EPC), np.float32)
        grow[rloc, eidx] = 1.0
        gcol[cloc, eidx] = 1.0
        # scatter matrix, edge-major blocks: S_res[p, s*128+n]
        smean = np.zeros((128, EPC), np.float32)
        sub = eidx // 128                         # global subchunk
        pp = eidx % 128
        smean[pp, sub * 128 + cloc] = invdeg[col_s[e0:e0 + EPC]]

        # selection (layer-6) structures
        sp = sel_pos[g0 * SEL_PG:(g0 + GPC) * SEL_PG] - e0    # local positions
        sg = sel_g[g0 * SEL_PG:(g0 + GPC) * SEL_PG] - g0      # local graph ids
        spair = sp // (2 * EPG)
        srow = row_s[e0 + sp] - n0 - spair * 128
        scol = col_s[e0 + sp] - n0 - spair * 128
        sidx = np.arange(SELC)
        assert np.array_equal(spair, sidx // 4), \
            "expected 2 selected edges per graph in order"
        g6r = np.zeros((128, SELC), np.float32)
        g6c = np.zeros((128, SELC), np.float32)
        g6r[srow, sidx] = 1.0
        g6c[scol, sidx] = 1.0
        ssel = np.zeros((128, GPC), np.float32)
        ssel[sidx, sg] = 1.0

        # e5 extraction metadata: per chunk list of (src_off, dst, count)
        ch = sp // CHUNK
        off = sp % CHUNK
        meta = []
        for c in range(NCH):
            m_ = np.where(ch == c)[0]
            if len(m_) == 0:
                continue
            o = off[m_]
            runs = []
            s = 0
            while s < len(m_):
                t = s
                while t + 1 < len(m_) and o[t + 1] == o[t] + 1:
                    t += 1
                runs.append((int(o[s]), int(m_[s]), t - s + 1))
                s = t + 1
            meta.append((c, runs))
        if sel_copy_meta is None:
            sel_copy_meta = meta
        else:
            assert sel_copy_meta == meta, "sel layout must match across cores"

        in_maps.append({
            'x_em': x_em.astype(nph),
            'e0_em': e_em.astype(nph),
            'grow': grow.astype(nph),
            'gcol': gcol.astype(nph),
            'smean': smean.astype(nph),
            'g6r': g6r.astype(nph),
            'g6c': g6c.astype(nph),
            'ssel': ssel.astype(nph),
            'w128': w128, 'w256': w256, 'w512': w512,
            'bias': biasT, 'ba3rep': ba3rep_h,
            'bn_g': bn_g, 'bn_b': bn_b, 'be_g': be_g, 'be_b': be_b,
        })
    return in_maps, pk, sel_copy_meta


# ---------------------------------------------------------------- device

def build_program(pk, sel_meta):
    nc = bacc.Bacc("TRN2", target_bir_lowering=False, debug=False,
                   num_devices=NCORES)

    def din(name, shape, dtype):
        return nc.dram_tensor(name, shape, dtype, kind="ExternalInput")

    x_em_d = din('x_em', [128, (NPC // 128) * NF], HDT)
    e0_em_d = din('e0_em', [128, (EPC // 128) * EF], HDT)
    grow_d = din('grow', [128, EPC], HDT)
    gcol_d = din('gcol', [128, EPC], HDT)
    smean_d = din('smean', [128, EPC], HDT)
    g6r_d = din('g6r', [128, SELC], HDT)
    g6c_d = din('g6c', [128, SELC], HDT)
    ssel_d = din('ssel', [128, GPC], HDT)
    w128_d = din('w128', [len(pk.w[128]), 128, 128], HDT)
    w256_d = din('w256', [len(pk.w[256]), 128, 256], HDT)
    w512_d = din('w512', [len(pk.w[512]), 128, 512], HDT)
    bias_d = din('bias', [128, len(pk.bias)], F32)
    ba3rep_d = din('ba3rep', [128, 5 * 256], F32)
    bn_g_d = din('bn_g', [1, NF], F32)
    bn_b_d = din('bn_b', [1, NF], F32)
    be_g_d = din('be_g', [1, EF], F32)
    be_b_d = din('be_b', [1, EF], F32)
    out_d = nc.dram_tensor('out', [1, GPC], F32, kind="ExternalOutput")

    with tile.TileContext(nc) as tc:
        with (
            tc.tile_pool(name="const", bufs=1) as cpool,
            tc.tile_pool(name="wts", bufs=2) as wpool,
            tc.tile_pool(name="io", bufs=3) as iopool,
            tc.tile_pool(name="act", bufs=2) as apool,
            tc.tile_pool(name="pair", bufs=2) as ppool,
            tc.tile_pool(name="zp", bufs=1) as zpool,
            tc.tile_pool(name="xf", bufs=2) as xpool,
            tc.tile_pool(name="ps512", bufs=6, space="PSUM") as ps512,
            tc.tile_pool(name="psS", bufs=2, space="PSUM") as psS,
            tc.tile_pool(name="dram", bufs=1, space="DRAM") as dpool,
        ):
            build_body(nc, tc, pk, sel_meta, locals())
    nc.compile()
    return nc


def build_body(nc, tc, pk, sel_meta, env):
    cpool, wpool, iopool, apool = env['cpool'], env['wpool'], env['iopool'], env['apool']
    ppool, zpool, xpool = env['ppool'], env['zpool'], env['xpool']
    ps512, psS, dpool = env['ps512'], env['psS'], env['dpool']
    grow_d, gcol_d, smean_d = env['grow_d'], env['gcol_d'], env['smean_d']
    w128_d, w256_d, w512_d, bias_d = env['w128_d'], env['w256_d'], env['w512_d'], env['bias_d']

    # ---------------- constants
    ident = cpool.tile([128, 128], HDT, tag="ident")
    make_identity(nc, ident[:])
    grow = cpool.tile([128, EPC], HDT, tag="grow")
    nc.sync.dma_start(grow[:], grow_d[:])
    smean = cpool.tile([128, EPC], HDT, tag="smean")
    nc.sync.dma_start(smean[:], smean_d[:])
    biasT = cpool.tile([128, len(pk.bias)], F32, tag="bias")
    nc.sync.dma_start(biasT[:], bias_d[:])
    ba3rep = cpool.tile([128, 5, 256], F32, tag="ba3rep")
    nc.sync.dma_start(ba3rep[:], env['ba3rep_d'][:].rearrange("p (l w) -> p l w", l=5))
    x_em = cpool.tile([128, (NPC // 128) * NF], HDT, tag="x_em")
    nc.sync.dma_start(x_em[:], env['x_em_d'][:])
    e0_em = cpool.tile([128, (EPC // 128) * EF], HDT, tag="e0_em")
    nc.sync.dma_start(e0_em[:], env['e0_em_d'][:])

    def bias_ap(key, blk, parts=128):
        s, nb = pk.bidx[key]
        assert blk < nb
        return biasT[0:parts, s + blk:s + blk + 1]

    # ---------------- BatchNorm statistics (partial sums + AllReduce)
    ones_bf = cpool.tile([128, 1], HDT, tag="ones_bf")
    nc.gpsimd.memset(ones_bf[:], 1.0)
    ones_f = cpool.tile([128, 1], F32, tag="ones_f")
    nc.gpsimd.memset(ones_f[:], 1.0)

    x2 = cpool.tile([128, (NPC // 128) * NF], F32, tag="x2")
    nc.vector.tensor_tensor(out=x2[:], in0=x_em[:], in1=x_em[:], op=ALU.mult)
    e2 = cpool.tile([128, (EPC // 128) * EF], F32, tag="e2")
    nc.vector.tensor_tensor(out=e2[:], in0=e0_em[:], in1=e0_em[:], op=ALU.mult)

    ps_xs = psS.tile([1, NF], F32, tag="psS")
    ps_x2 = psS.tile([1, NF], F32, tag="psS")
    for t in range(NPC // 128):
        nc.tensor.matmul(ps_xs[:], lhsT=ones_bf[:], rhs=x_em[:, t * NF:(t + 1) * NF],
                         start=(t == 0), stop=(t == NPC // 128 - 1))
    for t in range(NPC // 128):
        nc.tensor.matmul(ps_x2[:], lhsT=ones_f[:], rhs=x2[:, t * NF:(t + 1) * NF],
                         start=(t == 0), stop=(t == NPC // 128 - 1))
    ps_es = psS.tile([1, EF], F32, tag="psS")
    ps_e2 = psS.tile([1, EF], F32, tag="psS")
    for t in range(EPC // 128):
        nc.tensor.matmul(ps_es[:], lhsT=ones_bf[:], rhs=e0_em[:, t * EF:(t + 1) * EF],
                         start=(t == 0), stop=(t == EPC // 128 - 1))
    for t in range(EPC // 128):
        nc.tensor.matmul(ps_e2[:], lhsT=ones_f[:], rhs=e2[:, t * EF:(t + 1) * EF],
                         start=(t == 0), stop=(t == EPC // 128 - 1))

    stat = cpool.tile([1, 96], F32, tag="stat")
    nc.vector.tensor_copy(stat[:, 0:32], ps_xs[:])
    nc.vector.tensor_copy(stat[:, 32:64], ps_x2[:])
    nc.vector.tensor_copy(stat[:, 64:80], ps_es[:])
    nc.vector.tensor_copy(stat[:, 80:96], ps_e2[:])

    cc_in = dpool.tile([1, 96], F32)
    cc_out = dpool.tile([1, 96], F32)
    nc.sync.dma_start(cc_in[:], stat[:])
    nc.gpsimd.collective_compute(
        "AllReduce", ALU.add, replica_groups=[list(range(NCORES))],
        ins=[cc_in.opt()], outs=[cc_out.opt()])
    statg = cpool.tile([1, 96], F32, tag="statg")
    nc.sync.dma_start(statg[:], cc_out[:])

    # affine params in free-dim layout, then bounce to partition layout
    aff = cpool.tile([1, 96], F32, tag="aff")
    tmp = cpool.tile([1, 512], F32, tag="bntmp")
    gparams = [env['bn_g_d'], env['bn_b_d'], env['be_g_d'], env['be_b_d']]
    gtiles = []
    for i, d in enumerate(gparams):
        t = cpool.tile([1, [NF, NF, EF, EF][i]], F32, tag=f"bnp{i}")
        nc.sync.dma_start(t[:], d[:])
        gtiles.append(t)
    for (s0, g_t, b_t, cnt, ntot, t0) in (
            (0, gtiles[0], gtiles[1], NF, float(N), 0),
            (64, gtiles[2], gtiles[3], EF, float(E), 256)):
        def seg(i):
            return tmp[:, t0 + i * cnt:t0 + (i + 1) * cnt]
        mean, ex2, m2, varr, std, rstd, ms, veps = (seg(i) for i in range(8))
        nc.vector.tensor_scalar_mul(mean, statg[:, s0:s0 + cnt], 1.0 / ntot)
        nc.vector.tensor_scalar_mul(ex2, statg[:, s0 + cnt:s0 + 2 * cnt], 1.0 / ntot)
        nc.vector.tensor_tensor(out=m2, in0=mean, in1=mean, op=ALU.mult)
        nc.vector.tensor_tensor(out=varr, in0=ex2, in1=m2, op=ALU.subtract)
        nc.vector.tensor_scalar_add(veps, varr, 1e-5)
        nc.scalar.activation(std, veps, ACTF.Sqrt)
        nc.vector.reciprocal(rstd, std)
        scale = aff[:, s0:s0 + cnt]
        nc.vector.tensor_tensor(out=scale, in0=g_t[:], in1=rstd, op=ALU.mult)
        shift = aff[:, s0 + cnt:s0 + 2 * cnt]
        nc.vector.tensor_tensor(out=ms, in0=mean, in1=scale, op=ALU.mult)
        nc.vector.tensor_tensor(out=shift, in0=b_t[:], in1=ms, op=ALU.subtract)
    cc_aff = dpool.tile([1, 96], F32)
    nc.sync.dma_start(cc_aff[:], aff[:])
    affp = []
    for i, (s0, cnt) in enumerate(((0, NF), (NF, NF), (64, EF), (64 + EF, EF))):
        t = cpool.tile([cnt, 1], F32, tag=f"afft{i}")
        nc.sync.dma_start(t[:], cc_aff[:, s0:s0 + cnt].rearrange("a b -> b a"))
        affp.append(t[:])
    sc_x, sh_x, sc_e, sh_e = affp

    # ---------------- x0_fm: transpose + BN affine, zero-padded to 128 rows
    x0_fm = cpool.tile([128, NPC], HDT, tag="x0fm")
    nc.gpsimd.memset(x0_fm[:], 0.0)
    for t in range(NPC // 128):
        pst = psS.tile([NF, 128], HDT, tag="psS")
        nc.tensor.transpose(pst[:], x_em[:, t * NF:(t + 1) * NF], ident[:])
        nc.vector.tensor_scalar(
            out=x0_fm[0:NF, t * 128:(t + 1) * 128], in0=pst[:],
            scalar1=sc_x, scalar2=sh_x, op0=ALU.mult, op1=ALU.add)

    # ---------------- edge DRAM ping-pong buffers
    eA = dpool.tile([NCH, 128, 4 * CHUNK], HDT)
    eB = dpool.tile([NCH, 128, 4 * CHUNK], HDT)

    def wslice(wtile, lstart, key, blk, width):
        w, s, nb = pk.idx[key]
        assert w == width and blk < nb
        return wtile[:, s - lstart + blk]

    e5sel = cpool.tile([128, 4, SELC], HDT, tag="e5sel")
    xfm_cur = x0_fm

    # ================= layers 1..5
    for l in range(1, 6):
        nfx = NF if l == 1 else 128
        s128, n128 = pk.layer_w[(l, 128)]
        s256, n256 = pk.layer_w[(l, 256)]
        s512, n512 = pk.layer_w[(l, 512)]
        wl128 = wpool.tile([128, n128, 128], HDT, tag="w128")
        nc.sync.dma_start(wl128[:], w128_d[s128:s128 + n128].rearrange("n p w -> p n w"))
        wl256 = wpool.tile([128, n256, 256], HDT, tag="w256")
        nc.sync.dma_start(wl256[:], w256_d[s256:s256 + n256].rearrange("n p w -> p n w"))
        wl512 = wpool.tile([128, n512, 512], HDT, tag="w512")
        nc.sync.dma_start(wl512[:], w512_d[s512:s512 + n512].rearrange("n p w -> p n w"))

        W = lambda key, blk=0, width=128: wslice(
            {128: wl128, 256: wl256, 512: wl512}[width],
            {128: s128, 256: s256, 512: s512}[width], key, blk, width)

        # ---- z projections (node-major), per pair
        z_row = zpool.tile([128, PAIRS * 128], HDT, tag="zrow")
        z_col = zpool.tile([128, PAIRS * 128], HDT, tag="zcol")
        z_a = zpool.tile([128, PAIRS * 256], HDT, tag="za")
        for p in range(PAIRS):
            xs = xfm_cur[:, p * 128:(p + 1) * 128]
            pz = psS.tile([128, 128], F32, tag="psS")
            nc.tensor.matmul(pz[:], lhsT=xs, rhs=W(f'e{l}r'), start=True, stop=True)
            nc.vector.tensor_copy(z_row[:, p * 128:(p + 1) * 128], pz[:])
            pz2 = psS.tile([128, 128], F32, tag="psS")
            nc.tensor.matmul(pz2[:], lhsT=xs, rhs=W(f'e{l}c'), start=True, stop=True)
            nc.vector.tensor_copy(z_col[:, p * 128:(p + 1) * 128], pz2[:])
            pz3 = psS.tile([128, 256], F32, tag="psS")
            nc.tensor.matmul(pz3[:], lhsT=xs, rhs=W(f'na{l}x', 0, 256), start=True, stop=True)
            nc.vector.tensor_copy(z_a[:, p * 256:(p + 1) * 256], pz3[:])

        xfm_next = xpool.tile([128, NPC], HDT, tag="xfm")
        m_em = None
        sel_meta_d = dict(sel_meta) if l == 5 else {}

        for c in range(NCH):
            p = c // 2
            # ---- edge features for this chunk (feature-major)
            if l == 1:
                e_prev = iopool.tile([128, CHUNK], HDT, tag="e0fm")
                nc.gpsimd.memset(e_prev[:], 0.0)
                for i in range(4):
                    u = c * 4 + i
                    pst = psS.tile([EF, 128], HDT, tag="psS")
                    nc.tensor.transpose(pst[:], e0_em[:, u * EF:(u + 1) * EF], ident[:])
                    nc.vector.tensor_scalar(
                        out=e_prev[0:EF, i * 128:(i + 1) * 128], in0=pst[:],
                        scalar1=sc_e, scalar2=sh_e, op0=ALU.mult, op1=ALU.add)
                ek = 1
                eblk = lambda k: e_prev[:]
            else:
                e_prev = iopool.tile([128, 4 * CHUNK], HDT, tag="eprev")
                src = eA if l % 2 == 0 else eB
                nc.sync.dma_start(e_prev[:], src[c])
                ek = 4
                eblk = lambda k: e_prev[:, k * CHUNK:(k + 1) * CHUNK]

            gr = grow[:, c * CHUNK:(c + 1) * CHUNK]
            gc_t = iopool.tile([128, CHUNK], HDT, tag="gcol")
            nc.sync.dma_start(gc_t[:], gcol_d[:, c * CHUNK:(c + 1) * CHUNK])

            # ---- edge MLP layer 1
            ph1 = ps512.tile([128, CHUNK], F32, tag="ps512")
            nc.tensor.matmul(ph1[:], lhsT=z_row[:, p * 128:(p + 1) * 128], rhs=gr,
                             start=True, stop=False)
            nc.tensor.matmul(ph1[:], lhsT=z_col[:, p * 128:(p + 1) * 128], rhs=gc_t[:],
                             start=False, stop=False)
            for k in range(ek):
                nc.tensor.matmul(ph1[:], lhsT=W(f'e{l}e', k), rhs=eblk(k),
                                 start=False, stop=(k == ek - 1))
            h1 = apool.tile([128, CHUNK], HDT, tag="h1")
            nc.scalar.activation(h1[:], ph1[:], ACTF.Relu, bias=bias_ap(f'e{l}1', 0))
            # ---- edge MLP layer 2
            ph2 = ps512.tile([128, CHUNK], F32, tag="ps512")
            nc.tensor.matmul(ph2[:], lhsT=W(f'e{l}2'), rhs=h1[:], start=True, stop=True)
            h2 = apool.tile([128, CHUNK], HDT, tag="h2")
            nc.scalar.activation(h2[:], ph2[:], ACTF.Relu, bias=bias_ap(f'e{l}2', 0))
            # ---- edge MLP layer 3 -> e_new (512 wide)
            e_new = iopool.tile([128, 4 * CHUNK], HDT, tag="enew")
            for k in range(4):
                pe = ps512.tile([128, CHUNK], F32, tag="ps512")
                nc.tensor.matmul(pe[:], lhsT=W(f'e{l}3', 0, 512)[:, k * 128:(k + 1) * 128],
                                 rhs=h2[:], start=True, stop=True)
                nc.vector.tensor_scalar_add(
                    e_new[:, k * CHUNK:(k + 1) * CHUNK], pe[:], bias_ap(f'e{l}3', k))
            if l < 5:
                dst = eB if l % 2 == 0 else eA
                if l == 1:
                    dst = eA
                nc.sync.dma_start(dst[c], e_new[:])
            if l == 5 and c in sel_meta_d:
                for (o, d0, cnt) in sel_meta_d[c]:
                    for k in range(4):
                        nc.vector.tensor_copy(
                            e5sel[:, k, d0:d0 + cnt],
                            e_new[:, k * CHUNK + o:k * CHUNK + o + cnt])

            # ---- node MLP a (layers 1-3), m in edge-major
            a1 = apool.tile([128, 2, CHUNK], HDT, tag="a1")
            for j in range(2):
                pa = ps512.tile([128, CHUNK], F32, tag="ps512")
                nc.tensor.matmul(pa[:], lhsT=z_a[:, p * 256 + j * 128:p * 256 + (j + 1) * 128],
                                 rhs=gr, start=True, stop=False)
                for k in range(4):
                    nc.tensor.matmul(pa[:], lhsT=W(f'na{l}e', k, 256)[:, j * 128:(j + 1) * 128],
                                     rhs=e_new[:, k * CHUNK:(k + 1) * CHUNK],
                                     start=False, stop=(k == 3))
                nc.scalar.activation(a1[:, j, :], pa[:], ACTF.Relu,
                                     bias=bias_ap(f'na{l}1', j))
            a2 = apool.tile([128, 2, CHUNK], HDT, tag="a2")
            for j in range(2):
                pa = ps512.tile([128, CHUNK], F32, tag="ps512")
                for k in range(2):
                    nc.tensor.matmul(pa[:], lhsT=W(f'na{l}2', k, 256)[:, j * 128:(j + 1) * 128],
                                     rhs=a1[:, k, :], start=(k == 0), stop=(k == 1))
                nc.scalar.activation(a2[:, j, :], pa[:], ACTF.Relu,
                                     bias=bias_ap(f'na{l}2', j))
            if c % 2 == 0:
                m_em = ppool.tile([128, 8, 256], HDT, tag="mem")
            for q in range(4):
                pm = psS.tile([128, 256], F32, tag="psS")
                for k in range(2):
                    nc.tensor.matmul(pm[:], lhsT=a2[:, k, q * 128:(q + 1) * 128],
                                     rhs=W(f'na{l}3', k, 256), start=(k == 0), stop=(k == 1))
                nc.vector.tensor_tensor(out=m_em[:, (c % 2) * 4 + q, :], in0=pm[:],
                                        in1=ba3rep[:, l - 1, :], op=ALU.add)

            if c % 2 == 1:
                # ---- scatter-mean (per pair), feature-major agg
                agg = ppool.tile([128, 2, 128], HDT, tag="agg")
                for j in range(2):
                    pg = psS.tile([128, 128], F32, tag="psS")
                    for q in range(8):
                        s = p * 8 + q
                        nc.tensor.matmul(pg[:], lhsT=m_em[:, q, j * 128:(j + 1) * 128],
                                         rhs=smean[:, s * 128:(s + 1) * 128],
                                         start=(q == 0), stop=(q == 7))
                    nc.vector.tensor_copy(agg[:, j, :], pg[:])
                # ---- node MLP b
                b1 = ppool.tile([128, 2, 128], HDT, tag="b1")
                for j in range(2):
                    pb = psS.tile([128, 128], F32, tag="psS")
                    nc.tensor.matmul(pb[:], lhsT=W(f'nb{l}x', 0, 256)[:, j * 128:(j + 1) * 128],
                                     rhs=xfm_cur[:, p * 128:(p + 1) * 128],
                                     start=True, stop=False)
                    for k in range(2):
                        nc.tensor.matmul(pb[:], lhsT=W(f'nb{l}a', k, 256)[:, j * 128:(j + 1) * 128],
                                         rhs=agg[:, k, :], start=False, stop=(k == 1))
                    nc.scalar.activation(b1[:, j, :], pb[:], ACTF.Relu,
                                         bias=bias_ap(f'nb{l}1', j))
                px = psS.tile([128, 128], F32, tag="psS")
                for k in range(2):
                    nc.tensor.matmul(px[:], lhsT=W(f'nb{l}2', k), rhs=b1[:, k, :],
                                     start=(k == 0), stop=(k == 1))
                nc.vector.tensor_scalar_add(
                    xfm_next[:, p * 128:(p + 1) * 128], px[:], bias_ap(f'nb{l}2', 0))
        xfm_cur = xfm_next

    # ================= layer 6 (selected edges only) + head
    l = 6
    s128, n128 = pk.layer_w[(6, 128)]
    sh128, nh128 = pk.layer_w[(7, 128)]
    wl128 = wpool.tile([128, n128 + nh128, 128], HDT, tag="w128")
    nc.sync.dma_start(wl128[:], w128_d[s128:s128 + n128 + nh128].rearrange("n p w -> p n w"))
    W6 = lambda key, blk=0: wslice(wl128, s128, key, blk, 128)

    g6r = cpool.tile([128, SELC], HDT, tag="g6r")
    nc.sync.dma_start(g6r[:], env['g6r_d'][:])
    g6c = cpool.tile([128, SELC], HDT, tag="g6c")
    nc.sync.dma_start(g6c[:], env['g6c_d'][:])
    ssel = cpool.tile([128, GPC], HDT, tag="ssel")
    nc.sync.dma_start(ssel[:], env['ssel_d'][:])

    z6r = zpool.tile([128, PAIRS * 128], HDT, tag="zrow")
    z6c = zpool.tile([128, PAIRS * 128], HDT, tag="zcol")
    for p in range(PAIRS):
        xs = xfm_cur[:, p * 128:(p + 1) * 128]
        pz = psS.tile([128, 128], F32, tag="psS")
        nc.tensor.matmul(pz[:], lhsT=xs, rhs=W6('e6r'), start=True, stop=True)
        nc.vector.tensor_copy(z6r[:, p * 128:(p + 1) * 128], pz[:])
        pz2 = psS.tile([128, 128], F32, tag="psS")
        nc.tensor.matmul(pz2[:], lhsT=xs, rhs=W6('e6c'), start=True, stop=True)
        nc.vector.tensor_copy(z6c[:, p * 128:(p + 1) * 128], pz2[:])

    pg6 = psS.tile([128, SELC], F32, tag="psS")
    for p in range(PAIRS):
        sl = slice(4 * p, 4 * p + 4)
        nc.tensor.matmul(pg6[:, sl], lhsT=z6r[:, p * 128:(p + 1) * 128],
                         rhs=g6r[:, sl], start=True, stop=False)
        nc.tensor.matmul(pg6[:, sl], lhsT=z6c[:, p * 128:(p + 1) * 128],
                         rhs=g6c[:, sl], start=False, stop=True)
    ph6 = psS.tile([128, SELC], F32, tag="psS")
    for k in range(4):
        nc.tensor.matmul(ph6[:], lhsT=W6('e6e', k), rhs=e5sel[:, k, :],
                         start=(k == 0), stop=(k == 3))
    ph6s = cpool.tile([128, SELC], F32, tag="ph6s")
    nc.vector.tensor_copy(ph6s[:], ph6[:])
    h6a = cpool.tile([128, SELC], HDT, tag="h6a")
    nc.vector.tensor_tensor(out=h6a[:], in0=pg6[:], in1=ph6s[:], op=ALU.add)
    h6 = cpool.tile([128, SELC], HDT, tag="h6")
    nc.scalar.activation(h6[:], h6a[:], ACTF.Relu, bias=bias_ap('e61', 0))
    ph7 = psS.tile([128, SELC], F32, tag="psS")
    nc.tensor.matmul(ph7[:], lhsT=W6('e62'), rhs=h6[:], start=True, stop=True)
    h7 = cpool.tile([128, SELC], HDT, tag="h7")
    nc.scalar.activation(h7[:], ph7[:], ACTF.Relu, bias=bias_ap('e62', 0))
    pe6 = psS.tile([128, SELC], F32, tag="psS")
    nc.tensor.matmul(pe6[:], lhsT=W6('e63'), rhs=h7[:], start=True, stop=True)
    e6f = cpool.tile([128, SELC], HDT, tag="e6f")
    nc.vector.tensor_scalar_add(e6f[:], pe6[:], bias_ap('e63', 0))

    # transpose to edge-major, pad, project per graph
    e6em = cpool.tile([128, 128], HDT, tag="e6em")
    nc.gpsimd.memset(e6em[:], 0.0)
    pt = psS.tile([SELC, 128], HDT, tag="psS")
    nc.tensor.transpose(pt[:], e6f[:], ident[:])
    nc.vector.tensor_copy(e6em[0:SELC, :], pt[:])
    py = psS.tile([128, GPC], F32, tag="psS")
    nc.tensor.matmul(py[:], lhsT=e6em[:], rhs=ssel[:], start=True, stop=True)
    ysb = cpool.tile([128, GPC], HDT, tag="ysb")
    nc.vector.tensor_copy(ysb[:], py[:])
    phh = psS.tile([128, GPC], F32, tag="psS")
    nc.tensor.matmul(phh[:], lhsT=W6('h1'), rhs=ysb[:], start=True, stop=True)
    hh = cpool.tile([128, GPC], HDT, tag="hh")
    nc.scalar.activation(hh[:], phh[:], ACTF.Relu, bias=bias_ap('h1', 0))
    po = psS.tile([1, GPC], F32, tag="psS")
    nc.tensor.matmul(po[:], lhsT=W6('h2')[:, 0:1], rhs=hh[:], start=True, stop=True)
    osb = cpool.tile([1, GPC], F32, tag="osb")
    nc.vector.tensor_scalar_add(osb[:], po[:], bias_ap('h2', 0, parts=1))
    nc.sync.dma_start(env['out_d'][:], osb[:])


def kernel(**inputs) -> np.ndarray:
    in_maps, pk, sel_meta = prep_inputs(inputs)
    key = 'prog'
    if key not in _CACHE:
        _CACHE[key] = build_program(pk, sel_meta)
    nc = _CACHE[key]
    res = run_bass_kernel_spmd(nc, in_maps, list(range(NCORES)))
    kernel.last_results = res
    out = np.concatenate([res.results[c]['out'].reshape(GPC) for c in range(NCORES)])
    return out.astype(np.float32).reshape(B, 1)


# revision 13
# speedup vs baseline: 1.1340x; 1.1340x over previous
"""Trainium2 Bass kernel for nn_Net_13486197310235 (GNN message passing).

Data-parallel over graphs: 8 cores x 32 graphs each. All MLP compute is done
as feature-major matmuls on the PE array in bf16 (fp32 PSUM accumulation).
Per-edge gathers x[row]/x[col] use the factored form (project nodes once,
then expand with per-graph-pair one-hot matmuls); scatter-mean uses one-hot
matmuls with 1/deg folded into the selection matrix. BatchNorm statistics are
computed on-device with a cross-core AllReduce.

Self-contained: hardcodes the problem shapes (B=256 graphs, 64 nodes/graph,
512 edges/graph, NF=32, EF=16).
"""

import numpy as np
import ml_dtypes

import concourse.bacc as bacc
import concourse.bass as bass
import concourse.mybir as mybir
import concourse.tile as tile
from concourse.bass_utils import run_bass_kernel_spmd
from concourse.masks import make_identity

NCORES = 8
B, NPG, EPG = 256, 64, 512
NF, EF = 32, 16
N, E = B * NPG, B * EPG

GPC = B // NCORES          # graphs per core = 32
NPC = GPC * NPG            # nodes per core = 2048
EPC = GPC * EPG            # edges per core = 16384
PAIRS = GPC // 2           # graph pairs per core = 16
CHUNK = 512                # edges per compute chunk
NCH = EPC // CHUNK         # chunks per core = 32
SEL_PG = 2                 # selected edges per graph
SELC = GPC * SEL_PG        # selected edges per core = 64

HDT = mybir.dt.float16
F32 = mybir.dt.float32
nph = np.float16

ALU = mybir.AluOpType
ACTF = mybir.ActivationFunctionType

_CACHE = {}


# ---------------------------------------------------------------- host pack

class Pack:
    """Accumulates weight blocks ([128, w] K-blocks) and bias blocks."""

    def __init__(self):
        self.w = {128: [], 256: [], 512: []}
        self.bias = []
        self.idx = {}       # key -> (width, start, nblocks)
        self.bidx = {}      # key -> (start, nblocks)
        self.layer_w = {}   # (layer, width) -> [start, count]

    def mark_layer(self, layer):
        self._layer = layer
        for w in (128, 256, 512):
            self.layer_w[(layer, w)] = [len(self.w[w]), 0]

    def add_w(self, key, W, width):
        W = np.asarray(W, np.float32)
        din, dout = W.shape
        assert dout <= width
        kb = -(-din // 128)
        Wp = np.zeros((kb * 128, width), np.float32)
        Wp[:din, :dout] = W
        start = len(self.w[width])
        for k in range(kb):
            self.w[width].append(Wp[k * 128:(k + 1) * 128])
        self.idx[key] = (width, start, kb)
        self.layer_w[(self._layer, width)][1] += kb

    def add_b(self, key, b):
        b = np.asarray(b, np.float32).reshape(-1)
        nb = -(-len(b) // 128)
        bp = np.zeros(nb * 128, np.float32)
        bp[:len(b)] = b
        start = len(self.bias)
        for k in range(nb):
            self.bias.append(bp[k * 128:(k + 1) * 128])
        self.bidx[key] = (start, nb)


def pack_weights(params):
    pk = Pack()
    for l in range(1, 7):
        pk.mark_layer(l)
        # edge MLP
        (W1, b1), (W2, b2), (W3, b3) = params[f'e{l}']
        nfx = NF if l == 1 else 128
        W1 = np.asarray(W1, np.float32)
        pk.add_w(f'e{l}r', W1[:nfx], 128)
        pk.add_w(f'e{l}c', W1[nfx:2 * nfx], 128)
        pk.add_w(f'e{l}e', W1[2 * nfx:], 128)
        pk.add_w(f'e{l}2', W2, 128)
        pk.add_w(f'e{l}3', W3, 512 if l < 6 else 128)
        pk.add_b(f'e{l}1', b1)
        pk.add_b(f'e{l}2', b2)
        pk.add_b(f'e{l}3', b3)
        if l == 6:
            break
        # node MLP a
        (Wa1, ba1), (Wa2, ba2), (Wa3, ba3) = params[f'n{l}a']
        Wa1 = np.asarray(Wa1, np.float32)
        pk.add_w(f'na{l}x', Wa1[:nfx], 256)
        pk.add_w(f'na{l}e', Wa1[nfx:], 256)
        pk.add_w(f'na{l}2', Wa2, 256)
        pk.add_w(f'na{l}3', Wa3, 256)
        pk.add_b(f'na{l}1', ba1)
        pk.add_b(f'na{l}2', ba2)
        # ba3 handled via replicated tile
        # node MLP b
        (Wb1, bb1), (Wb2, bb2) = params[f'n{l}b']
        Wb1 = np.asarray(Wb1, np.float32)
        pk.add_w(f'nb{l}x', Wb1[:nfx], 256)
        pk.add_w(f'nb{l}a', Wb1[nfx:], 256)
        pk.add_w(f'nb{l}2', Wb2, 128)
        pk.add_b(f'nb{l}1', bb1)
        pk.add_b(f'nb{l}2', bb2)
    pk.mark_layer(7)
    (Wh1, bh1), (Wh2, bh2) = params['head']
    pk.add_w('h1', Wh1, 128)
    pk.add_w('h2', Wh2, 128)
    pk.add_b('h1', bh1)
    pk.add_b('h2', bh2)

    ba3rep = np.zeros((128, 5, 256), np.float32)
    for l in range(1, 6):
        ba3rep[:, l - 1, :] = np.asarray(params[f'n{l}a'][2][1], np.float32)[None, :]
    return pk, ba3rep


# ---------------------------------------------------------------- host shard

def prep_inputs(inputs):
    """Shard + reorder + build one-hot matrices. Returns per-core input maps
    plus the pack object (weights are replicated)."""
    params = inputs['params']
    x = np.asarray(inputs['x'], np.float32)
    ea = np.asarray(inputs['edge_attr'], np.float32)
    ei = np.asarray(inputs['edge_index'], np.int64)
    batch = np.asarray(inputs['batch'], np.int64)
    sel_idx = np.asarray(inputs['sel_idx'], np.int64)
    eb = np.asarray(inputs['eb'], np.int64)

    row, col = ei[0], ei[1]
    gid = batch[row]
    assert np.array_equal(batch[col], gid), "edges must stay within one graph"
    # group edges by graph (stable) -> per-graph contiguous blocks of EPG
    perm = np.argsort(gid, kind='stable')
    counts = np.bincount(gid, minlength=B)
    assert np.all(counts == EPG), "expected uniform edges/graph"
    assert np.array_equal(batch, np.repeat(np.arange(B), NPG)), \
        "expected uniform nodes/graph"
    row_s, col_s, ea_s = row[perm], col[perm], ea[perm]
    inv = np.empty(E, np.int64)
    inv[perm] = np.arange(E)
    sel_pos = inv[sel_idx]            # position in reordered edge list
    # selected edges sorted by graph
    sel_g = eb
    assert np.all(np.bincount(sel_g, minlength=B) == SEL_PG)
    sord = np.argsort(sel_g, kind='stable')
    sel_pos = sel_pos[sord]
    sel_g = sel_g[sord]

    # in-degree over col (per node), computed per graph block
    deg = np.bincount(col_s, minlength=N).astype(np.float32)
    invdeg = 1.0 / np.maximum(deg, 1.0)

    pk, ba3rep = pack_weights(params)
    w128 = np.stack(pk.w[128]).astype(nph)
    w256 = np.stack(pk.w[256]).astype(nph)
    w512 = np.stack(pk.w[512]).astype(nph)
    biasT = np.stack(pk.bias, axis=1).astype(np.float32)     # [128, NB]
    ba3rep_h = ba3rep.reshape(128, 5 * 256)

    bn_g = np.asarray(params['bn_node'][0], np.float32)[None, :]
    bn_b = np.asarray(params['bn_node'][1], np.float32)[None, :]
    be_g = np.asarray(params['bn_edge'][0], np.float32)[None, :]
    be_b = np.asarray(params['bn_edge'][1], np.float32)[None, :]

    in_maps = []
    sel_copy_meta = None
    for cidx in range(NCORES):
        g0 = cidx * GPC
        n0, e0 = g0 * NPG, g0 * EPG
        xs = x[n0:n0 + NPC]                       # [2048, 32]
        eas = ea_s[e0:e0 + EPC]                   # [16384, 16]
        rows = row_s[e0:e0 + EPC] - n0            # local node idx 0..2047
        cols = col_s[e0:e0 + EPC] - n0

        # em layouts with partition dim = row-within-128-tile
        x_em = xs.reshape(NPC // 128, 128, NF).transpose(1, 0, 2).reshape(128, -1)
        e_em = eas.reshape(EPC // 128, 128, EF).transpose(1, 0, 2).reshape(128, -1)

        eidx = np.arange(EPC)
        pair = eidx // (2 * EPG)                  # pair of each edge
        rloc = rows - pair * 128                  # node idx within pair: 0..127
        cloc = cols - pair * 128
        assert rloc.min() >= 0 and rloc.max() < 128
        grow = np.zeros((128, EPC), np.float32)
        gcol = np.zeros((128, EPC), np.float32)
        grow[rloc, eidx] = 1.0
        gcol[cloc, eidx] = 1.0
        # scatter matrix, edge-major blocks: S_res[p, s*128+n]
        smean = np.zeros((128, EPC), np.float32)
        sub = eidx // 128                         # global subchunk
        pp = eidx % 128
        smean[pp, sub * 128 + cloc] = invdeg[col_s[e0:e0 + EPC]]

        # selection (layer-6) structures
        sp = sel_pos[g0 * SEL_PG:(g0 + GPC) * SEL_PG] - e0    # local positions
        sg = sel_g[g0 * SEL_PG:(g0 + GPC) * SEL_PG] - g0      # local graph ids
        spair = sp // (2 * EPG)
        srow = row_s[e0 + sp] - n0 - spair * 128
        scol = col_s[e0 + sp] - n0 - spair * 128
        sidx = np.arange(SELC)
        assert np.array_equal(spair, sidx // 4), \
            "expected 2 selected edges per graph in order"
        g6r = np.zeros((128, SELC), np.float32)
        g6c = np.zeros((128, SELC), np.float32)
        g6r[srow, sidx] = 1.0
        g6c[scol, sidx] = 1.0
        ssel = np.zeros((128, GPC), np.float32)
        ssel[sidx, sg] = 1.0

        # e5 extraction metadata: per chunk list of (src_off, dst, count)
        ch = sp // CHUNK
        off = sp % CHUNK
        meta = []
        for c in range(NCH):
            m_ = np.where(ch == c)[0]
            if len(m_) == 0:
                continue
            o = off[m_]
            runs = []
            s = 0
            while s < len(m_):
                t = s
                while t + 1 < len(m_) and o[t + 1] == o[t] + 1:
                    t += 1
                runs.append((int(o[s]), int(m_[s]), t - s + 1))
                s = t + 1
            meta.append((c, runs))
        if sel_copy_meta is None:
            sel_copy_meta = meta
        else:
            assert sel_copy_meta == meta, "sel layout must match across cores"

        in_maps.append({
            'x_em': x_em.astype(nph),
            'e0_em': e_em.astype(nph),
            'grow': grow.astype(nph),
            'gcol': gcol.astype(nph),
            'smean': smean.astype(nph),
            'g6r': g6r.astype(nph),
            'g6c': g6c.astype(nph),
            'ssel': ssel.astype(nph),
            'w128': w128, 'w256': w256, 'w512': w512,
            'bias': biasT, 'ba3rep': ba3rep_h,
            'bn_g': bn_g, 'bn_b': bn_b, 'be_g': be_g, 'be_b': be_b,
        })
    return in_maps, pk, sel_copy_meta


# ---------------------------------------------------------------- device

def build_program(pk, sel_meta):
    nc = bacc.Bacc("TRN2", target_bir_lowering=False, debug=False,
                   num_devices=NCORES)

    def din(name, shape, dtype):
        return nc.dram_tensor(name, shape, dtype, kind="ExternalInput")

    x_em_d = din('x_em', [128, (NPC // 128) * NF], HDT)
    e0_em_d = din('e0_em', [128, (EPC // 128) * EF], HDT)
    grow_d = din('grow', [128, EPC], HDT)
    gcol_d = din('gcol', [128, EPC], HDT)
    smean_d = din('smean', [128, EPC], HDT)
    g6r_d = din('g6r', [128, SELC], HDT)
    g6c_d = din('g6c', [128, SELC], HDT)
    ssel_d = din('ssel', [128, GPC], HDT)
    w128_d = din('w128', [len(pk.w[128]), 128, 128], HDT)
    w256_d = din('w256', [len(pk.w[256]), 128, 256], HDT)
    w512_d = din('w512', [len(pk.w[512]), 128, 512], HDT)
    bias_d = din('bias', [128, len(pk.bias)], F32)
    ba3rep_d = din('ba3rep', [128, 5 * 256], F32)
    bn_g_d = din('bn_g', [1, NF], F32)
    bn_b_d = din('bn_b', [1, NF], F32)
    be_g_d = din('be_g', [1, EF], F32)
    be_b_d = din('be_b', [1, EF], F32)
    out_d = nc.dram_tensor('out', [1, GPC], F32, kind="ExternalOutput")

    with tile.TileContext(nc) as tc:
        with (
            tc.tile_pool(name="const", bufs=1) as cpool,
            tc.tile_pool(name="wts", bufs=2) as wpool,
            tc.tile_pool(name="io", bufs=3) as iopool,
            tc.tile_pool(name="act", bufs=3) as apool,
            tc.tile_pool(name="pair", bufs=3) as ppool,
            tc.tile_pool(name="zp", bufs=1) as zpool,
            tc.tile_pool(name="xf", bufs=2) as xpool,
            tc.tile_pool(name="ps512", bufs=5, space="PSUM") as ps512,
            tc.tile_pool(name="psS", bufs=3, space="PSUM") as psS,
            tc.tile_pool(name="dram", bufs=1, space="DRAM") as dpool,
        ):
            build_body(nc, tc, pk, sel_meta, locals())
    nc.compile()
    return nc


def build_body(nc, tc, pk, sel_meta, env):
    cpool, wpool, iopool, apool = env['cpool'], env['wpool'], env['iopool'], env['apool']
    ppool, zpool, xpool = env['ppool'], env['zpool'], env['xpool']
    ps512, psS, dpool = env['ps512'], env['psS'], env['dpool']
    grow_d, gcol_d, smean_d = env['grow_d'], env['gcol_d'], env['smean_d']
    w128_d, w256_d, w512_d, bias_d = env['w128_d'], env['w256_d'], env['w512_d'], env['bias_d']

    # ---------------- constants
    ident = cpool.tile([128, 128], HDT, tag="ident")
    make_identity(nc, ident[:])
    grow = cpool.tile([128, EPC], HDT, tag="grow")
    nc.sync.dma_start(grow[:], grow_d[:])
    smean = cpool.tile([128, EPC], HDT, tag="smean")
    nc.sync.dma_start(smean[:], smean_d[:])
    biasT = cpool.tile([128, len(pk.bias)], F32, tag="bias")
    nc.sync.dma_start(biasT[:], bias_d[:])
    ba3rep = cpool.tile([128, 5, 256], F32, tag="ba3rep")
    nc.sync.dma_start(ba3rep[:], env['ba3rep_d'][:].rearrange("p (l w) -> p l w", l=5))
    x_em = cpool.tile([128, (NPC // 128) * NF], HDT, tag="x_em")
    nc.sync.dma_start(x_em[:], env['x_em_d'][:])
    e0_em = cpool.tile([128, (EPC // 128) * EF], HDT, tag="e0_em")
    nc.sync.dma_start(e0_em[:], env['e0_em_d'][:])

    def bias_ap(key, blk, parts=128):
        s, nb = pk.bidx[key]
        assert blk < nb
        return biasT[0:parts, s + blk:s + blk + 1]

    # ---------------- BatchNorm statistics (partial sums + AllReduce)
    ones_bf = cpool.tile([128, 1], HDT, tag="ones_bf")
    nc.gpsimd.memset(ones_bf[:], 1.0)
    ones_f = cpool.tile([128, 1], F32, tag="ones_f")
    nc.gpsimd.memset(ones_f[:], 1.0)

    x2 = cpool.tile([128, (NPC // 128) * NF], F32, tag="x2")
    nc.vector.tensor_tensor(out=x2[:], in0=x_em[:], in1=x_em[:], op=ALU.mult)

    ps_xs = psS.tile([1, NF], F32, tag="psS")
    ps_x2 = psS.tile([1, NF], F32, tag="psS")
    for t in range(NPC // 128):
        nc.tensor.matmul(ps_xs[:], lhsT=ones_bf[:], rhs=x_em[:, t * NF:(t + 1) * NF],
                         start=(t == 0), stop=(t == NPC // 128 - 1))
    for t in range(NPC // 128):
        nc.tensor.matmul(ps_x2[:], lhsT=ones_f[:], rhs=x2[:, t * NF:(t + 1) * NF],
                         start=(t == 0), stop=(t == NPC // 128 - 1))
    ps_es = psS.tile([1, EF], F32, tag="psS")
    ps_e2 = psS.tile([1, EF], F32, tag="psS")
    for t in range(EPC // 128):
        nc.tensor.matmul(ps_es[:], lhsT=ones_bf[:], rhs=e0_em[:, t * EF:(t + 1) * EF],
                         start=(t == 0), stop=(t == EPC // 128 - 1))
    ntile = EPC // 128            # 128 tiles of EF cols
    for g in range(4):            # square in 4 column groups to save SBUF
        e2 = iopool.tile([128, (ntile // 4) * EF], F32, tag="e2chunk")
        base = g * (ntile // 4)
        nc.vector.tensor_tensor(
            out=e2[:], in0=e0_em[:, base * EF:(base + ntile // 4) * EF],
            in1=e0_em[:, base * EF:(base + ntile // 4) * EF], op=ALU.mult)
        for t in range(ntile // 4):
            gt = base + t
            nc.tensor.matmul(ps_e2[:], lhsT=ones_f[:], rhs=e2[:, t * EF:(t + 1) * EF],
                             start=(gt == 0), stop=(gt == ntile - 1))

    stat = cpool.tile([1, 96], F32, tag="stat")
    nc.vector.tensor_copy(stat[:, 0:32], ps_xs[:])
    nc.vector.tensor_copy(stat[:, 32:64], ps_x2[:])
    nc.vector.tensor_copy(stat[:, 64:80], ps_es[:])
    nc.vector.tensor_copy(stat[:, 80:96], ps_e2[:])

    cc_in = dpool.tile([1, 96], F32)
    cc_out = dpool.tile([1, 96], F32)
    nc.sync.dma_start(cc_in[:], stat[:])
    nc.gpsimd.collective_compute(
        "AllReduce", ALU.add, replica_groups=[list(range(NCORES))],
        ins=[cc_in.opt()], outs=[cc_out.opt()])
    statg = cpool.tile([1, 96], F32, tag="statg")
    nc.sync.dma_start(statg[:], cc_out[:])

    # affine params in free-dim layout, then bounce to partition layout
    aff = cpool.tile([1, 96], F32, tag="aff")
    tmp = cpool.tile([1, 512], F32, tag="bntmp")
    gparams = [env['bn_g_d'], env['bn_b_d'], env['be_g_d'], env['be_b_d']]
    gtiles = []
    for i, d in enumerate(gparams):
        t = cpool.tile([1, [NF, NF, EF, EF][i]], F32, tag=f"bnp{i}")
        nc.sync.dma_start(t[:], d[:])
        gtiles.append(t)
    for (s0, g_t, b_t, cnt, ntot, t0) in (
            (0, gtiles[0], gtiles[1], NF, float(N), 0),
            (64, gtiles[2], gtiles[3], EF, float(E), 256)):
        def seg(i):
            return tmp[:, t0 + i * cnt:t0 + (i + 1) * cnt]
        mean, ex2, m2, varr, std, rstd, ms, veps = (seg(i) for i in range(8))
        nc.vector.tensor_scalar_mul(mean, statg[:, s0:s0 + cnt], 1.0 / ntot)
        nc.vector.tensor_scalar_mul(ex2, statg[:, s0 + cnt:s0 + 2 * cnt], 1.0 / ntot)
        nc.vector.tensor_tensor(out=m2, in0=mean, in1=mean, op=ALU.mult)
        nc.vector.tensor_tensor(out=varr, in0=ex2, in1=m2, op=ALU.subtract)
        nc.vector.tensor_scalar_add(veps, varr, 1e-5)
        nc.scalar.activation(std, veps, ACTF.Sqrt)
        nc.vector.reciprocal(rstd, std)
        scale = aff[:, s0:s0 + cnt]
        nc.vector.tensor_tensor(out=scale, in0=g_t[:], in1=rstd, op=ALU.mult)
        shift = aff[:, s0 + cnt:s0 + 2 * cnt]
        nc.vector.tensor_tensor(out=ms, in0=mean, in1=scale, op=ALU.mult)
        nc.vector.tensor_tensor(out=shift, in0=b_t[:], in1=ms, op=ALU.subtract)
    cc_aff = dpool.tile([1, 96], F32)
    nc.sync.dma_start(cc_aff[:], aff[:])
    affp = []
    for i, (s0, cnt) in enumerate(((0, NF), (NF, NF), (64, EF), (64 + EF, EF))):
        t = cpool.tile([cnt, 1], F32, tag=f"afft{i}")
        nc.sync.dma_start(t[:], cc_aff[:, s0:s0 + cnt].rearrange("a b -> b a"))
        affp.append(t[:])
    sc_x, sh_x, sc_e, sh_e = affp

    # ---------------- x0_fm: transpose + BN affine, zero-padded to 128 rows
    x0_fm = cpool.tile([128, NPC], HDT, tag="x0fm")
    nc.gpsimd.memset(x0_fm[:], 0.0)
    for t in range(NPC // 128):
        pst = psS.tile([NF, 128], HDT, tag="psS")
        nc.tensor.transpose(pst[:], x_em[:, t * NF:(t + 1) * NF], ident[:])
        nc.vector.tensor_scalar(
            out=x0_fm[0:NF, t * 128:(t + 1) * 128], in0=pst[:],
            scalar1=sc_x, scalar2=sh_x, op0=ALU.mult, op1=ALU.add)

    # ---------------- edge DRAM ping-pong buffers
    eA = dpool.tile([NCH, 128, 4 * CHUNK], HDT)
    eB = dpool.tile([NCH, 128, 4 * CHUNK], HDT)

    def wslice(wtile, lstart, key, blk, width):
        w, s, nb = pk.idx[key]
        assert w == width and blk < nb
        return wtile[:, s - lstart + blk]

    e5sel = cpool.tile([128, 4, SELC], HDT, tag="e5sel")
    xfm_cur = x0_fm

    # ================= layers 1..5
    for l in range(1, 6):
        nfx = NF if l == 1 else 128
        s128, n128 = pk.layer_w[(l, 128)]
        s256, n256 = pk.layer_w[(l, 256)]
        s512, n512 = pk.layer_w[(l, 512)]
        wl128 = wpool.tile([128, n128, 128], HDT, tag="w128")
        nc.sync.dma_start(wl128[:], w128_d[s128:s128 + n128].rearrange("n p w -> p n w"))
        wl256 = wpool.tile([128, n256, 256], HDT, tag="w256")
        nc.sync.dma_start(wl256[:], w256_d[s256:s256 + n256].rearrange("n p w -> p n w"))
        wl512 = wpool.tile([128, n512, 512], HDT, tag="w512")
        nc.sync.dma_start(wl512[:], w512_d[s512:s512 + n512].rearrange("n p w -> p n w"))

        W = lambda key, blk=0, width=128: wslice(
            {128: wl128, 256: wl256, 512: wl512}[width],
            {128: s128, 256: s256, 512: s512}[width], key, blk, width)

        # ---- z projections (node-major), per pair
        z_row = zpool.tile([128, PAIRS * 128], HDT, tag="zrow")
        z_col = zpool.tile([128, PAIRS * 128], HDT, tag="zcol")
        z_a = zpool.tile([128, PAIRS * 256], HDT, tag="za")
        for p in range(PAIRS):
            xs = xfm_cur[:, p * 128:(p + 1) * 128]
            pz = psS.tile([128, 128], F32, tag="psS")
            nc.tensor.matmul(pz[:], lhsT=xs, rhs=W(f'e{l}r'), start=True, stop=True)
            nc.vector.tensor_copy(z_row[:, p * 128:(p + 1) * 128], pz[:])
            pz2 = psS.tile([128, 128], F32, tag="psS")
            nc.tensor.matmul(pz2[:], lhsT=xs, rhs=W(f'e{l}c'), start=True, stop=True)
            nc.vector.tensor_copy(z_col[:, p * 128:(p + 1) * 128], pz2[:])
            pz3 = psS.tile([128, 256], F32, tag="psS")
            nc.tensor.matmul(pz3[:], lhsT=xs, rhs=W(f'na{l}x', 0, 256), start=True, stop=True)
            nc.vector.tensor_copy(z_a[:, p * 256:(p + 1) * 256], pz3[:])

        xfm_next = xpool.tile([128, NPC], HDT, tag="xfm")
        m_em = None
        sel_meta_d = dict(sel_meta) if l == 5 else {}

        for c in range(NCH):
            p = c // 2
            # ---- edge features for this chunk (feature-major)
            if l == 1:
                e_prev = iopool.tile([128, CHUNK], HDT, tag="e0fm")
                nc.gpsimd.memset(e_prev[:], 0.0)
                for i in range(4):
                    u = c * 4 + i
                    pst = psS.tile([EF, 128], HDT, tag="psS")
                    nc.tensor.transpose(pst[:], e0_em[:, u * EF:(u + 1) * EF], ident[:])
                    nc.vector.tensor_scalar(
                        out=e_prev[0:EF, i * 128:(i + 1) * 128], in0=pst[:],
                        scalar1=sc_e, scalar2=sh_e, op0=ALU.mult, op1=ALU.add)
                ek = 1
                eblk = lambda k: e_prev[:]
            else:
                e_prev = iopool.tile([128, 4 * CHUNK], HDT, tag="eprev")
                src = eA if l % 2 == 0 else eB
                nc.sync.dma_start(e_prev[:], src[c])
                ek = 4
                eblk = lambda k: e_prev[:, k * CHUNK:(k + 1) * CHUNK]

            gr = grow[:, c * CHUNK:(c + 1) * CHUNK]
            gc_t = iopool.tile([128, CHUNK], HDT, tag="gcol")
            nc.sync.dma_start(gc_t[:], gcol_d[:, c * CHUNK:(c + 1) * CHUNK])

            # ---- edge MLP layer 1
            ph1 = ps512.tile([128, CHUNK], F32, tag="ps512")
            nc.tensor.matmul(ph1[:], lhsT=z_row[:, p * 128:(p + 1) * 128], rhs=gr,
                             start=True, stop=False)
            nc.tensor.matmul(ph1[:], lhsT=z_col[:, p * 128:(p + 1) * 128], rhs=gc_t[:],
                             start=False, stop=False)
            for k in range(ek):
                nc.tensor.matmul(ph1[:], lhsT=W(f'e{l}e', k), rhs=eblk(k),
                                 start=False, stop=(k == ek - 1))
            h1 = apool.tile([128, CHUNK], HDT, tag="h1")
            nc.scalar.activation(h1[:], ph1[:], ACTF.Relu, bias=bias_ap(f'e{l}1', 0))
            # ---- edge MLP layer 2
            ph2 = ps512.tile([128, CHUNK], F32, tag="ps512")
            nc.tensor.matmul(ph2[:], lhsT=W(f'e{l}2'), rhs=h1[:], start=True, stop=True)
            h2 = apool.tile([128, CHUNK], HDT, tag="h2")
            nc.scalar.activation(h2[:], ph2[:], ACTF.Relu, bias=bias_ap(f'e{l}2', 0))
            # ---- edge MLP layer 3 -> e_new (512 wide)
            e_new = iopool.tile([128, 4 * CHUNK], HDT, tag="enew")
            for k in range(4):
                pe = ps512.tile([128, CHUNK], F32, tag="ps512")
                nc.tensor.matmul(pe[:], lhsT=W(f'e{l}3', 0, 512)[:, k * 128:(k + 1) * 128],
                                 rhs=h2[:], start=True, stop=True)
                nc.vector.tensor_scalar_add(
                    e_new[:, k * CHUNK:(k + 1) * CHUNK], pe[:], bias_ap(f'e{l}3', k))
            if l < 5:
                dst = eB if l % 2 == 0 else eA
                if l == 1:
                    dst = eA
                nc.sync.dma_start(dst[c], e_new[:])
            if l == 5 and c in sel_meta_d:
                for (o, d0, cnt) in sel_meta_d[c]:
                    for k in range(4):
                        nc.vector.tensor_copy(
                            e5sel[:, k, d0:d0 + cnt],
                            e_new[:, k * CHUNK + o:k * CHUNK + o + cnt])

            # ---- node MLP a (layers 1-3), m in edge-major
            a1 = apool.tile([128, 2, CHUNK], HDT, tag="a1")
            for j in range(2):
                pa = ps512.tile([128, CHUNK], F32, tag="ps512")
                nc.tensor.matmul(pa[:], lhsT=z_a[:, p * 256 + j * 128:p * 256 + (j + 1) * 128],
                                 rhs=gr, start=True, stop=False)
                for k in range(4):
                    nc.tensor.matmul(pa[:], lhsT=W(f'na{l}e', k, 256)[:, j * 128:(j + 1) * 128],
                                     rhs=e_new[:, k * CHUNK:(k + 1) * CHUNK],
                                     start=False, stop=(k == 3))
                nc.scalar.activation(a1[:, j, :], pa[:], ACTF.Relu,
                                     bias=bias_ap(f'na{l}1', j))
            a2 = apool.tile([128, 2, CHUNK], HDT, tag="a2")
            for j in range(2):
                pa = ps512.tile([128, CHUNK], F32, tag="ps512")
                for k in range(2):
                    nc.tensor.matmul(pa[:], lhsT=W(f'na{l}2', k, 256)[:, j * 128:(j + 1) * 128],
                                     rhs=a1[:, k, :], start=(k == 0), stop=(k == 1))
                nc.scalar.activation(a2[:, j, :], pa[:], ACTF.Relu,
                                     bias=bias_ap(f'na{l}2', j))
            if c % 2 == 0:
                m_em = ppool.tile([128, 8, 256], HDT, tag="mem")
            for q in range(4):
                pm = psS.tile([128, 256], F32, tag="psS")
                for k in range(2):
                    nc.tensor.matmul(pm[:], lhsT=a2[:, k, q * 128:(q + 1) * 128],
                                     rhs=W(f'na{l}3', k, 256), start=(k == 0), stop=(k == 1))
                nc.vector.tensor_tensor(out=m_em[:, (c % 2) * 4 + q, :], in0=pm[:],
                                        in1=ba3rep[:, l - 1, :], op=ALU.add)

            if c % 2 == 1:
                # ---- scatter-mean (per pair), feature-major agg
                agg = ppool.tile([128, 2, 128], HDT, tag="agg")
                for j in range(2):
                    pg = psS.tile([128, 128], F32, tag="psS")
                    for q in range(8):
                        s = p * 8 + q
                        nc.tensor.matmul(pg[:], lhsT=m_em[:, q, j * 128:(j + 1) * 128],
                                         rhs=smean[:, s * 128:(s + 1) * 128],
                                         start=(q == 0), stop=(q == 7))
                    nc.vector.tensor_copy(agg[:, j, :], pg[:])
                # ---- node MLP b
                b1 = ppool.tile([128, 2, 128], HDT, tag="b1")
                for j in range(2):
                    pb = psS.tile([128, 128], F32, tag="psS")
                    nc.tensor.matmul(pb[:], lhsT=W(f'nb{l}x', 0, 256)[:, j * 128:(j + 1) * 128],
                                     rhs=xfm_cur[:, p * 128:(p + 1) * 128],
                                     start=True, stop=False)
                    for k in range(2):
                        nc.tensor.matmul(pb[:], lhsT=W(f'nb{l}a', k, 256)[:, j * 128:(j + 1) * 128],
                                         rhs=agg[:, k, :], start=False, stop=(k == 1))
                    nc.scalar.activation(b1[:, j, :], pb[:], ACTF.Relu,
                                         bias=bias_ap(f'nb{l}1', j))
                px = psS.tile([128, 128], F32, tag="psS")
                for k in range(2):
                    nc.tensor.matmul(px[:], lhsT=W(f'nb{l}2', k), rhs=b1[:, k, :],
                                     start=(k == 0), stop=(k == 1))
                nc.vector.tensor_scalar_add(
                    xfm_next[:, p * 128:(p + 1) * 128], px[:], bias_ap(f'nb{l}2', 0))
        xfm_cur = xfm_next

    # ================= layer 6 (selected edges only) + head
    l = 6
    s128, n128 = pk.layer_w[(6, 128)]
    sh128, nh128 = pk.layer_w[(7, 128)]
    wl128 = wpool.tile([128, n128 + nh128, 128], HDT, tag="w128")
    nc.sync.dma_start(wl128[:], w128_d[s128:s128 + n128 + nh128].rearrange("n p w -> p n w"))
    W6 = lambda key, blk=0: wslice(wl128, s128, key, blk, 128)

    g6r = cpool.tile([128, SELC], HDT, tag="g6r")
    nc.sync.dma_start(g6r[:], env['g6r_d'][:])
    g6c = cpool.tile([128, SELC], HDT, tag="g6c")
    nc.sync.dma_start(g6c[:], env['g6c_d'][:])
    ssel = cpool.tile([128, GPC], HDT, tag="ssel")
    nc.sync.dma_start(ssel[:], env['ssel_d'][:])

    z6r = zpool.tile([128, PAIRS * 128], HDT, tag="zrow")
    z6c = zpool.tile([128, PAIRS * 128], HDT, tag="zcol")
    for p in range(PAIRS):
        xs = xfm_cur[:, p * 128:(p + 1) * 128]
        pz = psS.tile([128, 128], F32, tag="psS")
        nc.tensor.matmul(pz[:], lhsT=xs, rhs=W6('e6r'), start=True, stop=True)
        nc.vector.tensor_copy(z6r[:, p * 128:(p + 1) * 128], pz[:])
        pz2 = psS.tile([128, 128], F32, tag="psS")
        nc.tensor.matmul(pz2[:], lhsT=xs, rhs=W6('e6c'), start=True, stop=True)
        nc.vector.tensor_copy(z6c[:, p * 128:(p + 1) * 128], pz2[:])

    pg6 = psS.tile([128, SELC], F32, tag="psS")
    for p in range(PAIRS):
        sl = slice(4 * p, 4 * p + 4)
        nc.tensor.matmul(pg6[:, sl], lhsT=z6r[:, p * 128:(p + 1) * 128],
                         rhs=g6r[:, sl], start=True, stop=False)
        nc.tensor.matmul(pg6[:, sl], lhsT=z6c[:, p * 128:(p + 1) * 128],
                         rhs=g6c[:, sl], start=False, stop=True)
    ph6 = psS.tile([128, SELC], F32, tag="psS")
    for k in range(4):
        nc.tensor.matmul(ph6[:], lhsT=W6('e6e', k), rhs=e5sel[:, k, :],
                         start=(k == 0), stop=(k == 3))
    ph6s = cpool.tile([128, SELC], F32, tag="ph6s")
    nc.vector.tensor_copy(ph6s[:], ph6[:])
    h6a = cpool.tile([128, SELC], HDT, tag="h6a")
    nc.vector.tensor_tensor(out=h6a[:], in0=pg6[:], in1=ph6s[:], op=ALU.add)
    h6 = cpool.tile([128, SELC], HDT, tag="h6")
    nc.scalar.activation(h6[:], h6a[:], ACTF.Relu, bias=bias_ap('e61', 0))
    ph7 = psS.tile([128, SELC], F32, tag="psS")
    nc.tensor.matmul(ph7[:], lhsT=W6('e62'), rhs=h6[:], start=True, stop=True)
    h7 = cpool.tile([128, SELC], HDT, tag="h7")
    nc.scalar.activation(h7[:], ph7[:], ACTF.Relu, bias=bias_ap('e62', 0))
    pe6 = psS.tile([128, SELC], F32, tag="psS")
    nc.tensor.matmul(pe6[:], lhsT=W6('e63'), rhs=h7[:], start=True, stop=True)
    e6f = cpool.tile([128, SELC], HDT, tag="e6f")
    nc.vector.tensor_scalar_add(e6f[:], pe6[:], bias_ap('e63', 0))

    # transpose to edge-major, pad, project per graph
    e6em = cpool.tile([128, 128], HDT, tag="e6em")
    nc.gpsimd.memset(e6em[:], 0.0)
    pt = psS.tile([SELC, 128], HDT, tag="psS")
    nc.tensor.transpose(pt[:], e6f[:], ident[:])
    nc.vector.tensor_copy(e6em[0:SELC, :], pt[:])
    py = psS.tile([128, GPC], F32, tag="psS")
    nc.tensor.matmul(py[:], lhsT=e6em[:], rhs=ssel[:], start=True, stop=True)
    ysb = cpool.tile([128, GPC], HDT, tag="ysb")
    nc.vector.tensor_copy(ysb[:], py[:])
    phh = psS.tile([128, GPC], F32, tag="psS")
    nc.tensor.matmul(phh[:], lhsT=W6('h1'), rhs=ysb[:], start=True, stop=True)
    hh = cpool.tile([128, GPC], HDT, tag="hh")
    nc.scalar.activation(hh[:], phh[:], ACTF.Relu, bias=bias_ap('h1', 0))
    po = psS.tile([1, GPC], F32, tag="psS")
    nc.tensor.matmul(po[:], lhsT=W6('h2')[:, 0:1], rhs=hh[:], start=True, stop=True)
    osb = cpool.tile([1, GPC], F32, tag="osb")
    nc.vector.tensor_scalar_add(osb[:], po[:], bias_ap('h2', 0, parts=1))
    nc.sync.dma_start(env['out_d'][:], osb[:])


def kernel(**inputs) -> np.ndarray:
    in_maps, pk, sel_meta = prep_inputs(inputs)
    key = 'prog'
    if key not in _CACHE:
        _CACHE[key] = build_program(pk, sel_meta)
    nc = _CACHE[key]
    res = run_bass_kernel_spmd(nc, in_maps, list(range(NCORES)))
    kernel.last_results = res
    out = np.concatenate([res.results[c]['out'].reshape(GPC) for c in range(NCORES)])
    return out.astype(np.float32).reshape(B, 1)


# revision 17
# speedup vs baseline: 1.1520x; 1.0158x over previous
"""Trainium2 Bass kernel for nn_Net_13486197310235 (GNN message passing).

Data-parallel over graphs: 8 cores x 32 graphs each. All MLP compute is done
as feature-major matmuls on the PE array in fp16 (fp32 PSUM accumulation).
Per-edge gathers x[row]/x[col] use the factored form (project nodes once,
then expand with per-graph-pair one-hot matmuls); scatter-mean uses one-hot
matmuls with 1/deg folded into the selection matrix. BatchNorm statistics are
computed on-device with a cross-core AllReduce.

Self-contained: hardcodes the problem shapes (B=256 graphs, 64 nodes/graph,
512 edges/graph, NF=32, EF=16).
"""

import numpy as np
import ml_dtypes

import concourse.bacc as bacc
import concourse.bass as bass
import concourse.mybir as mybir
import concourse.tile as tile
from concourse.bass_utils import run_bass_kernel_spmd
from concourse.masks import make_identity

NCORES = 8
B, NPG, EPG = 256, 64, 512
NF, EF = 32, 16
N, E = B * NPG, B * EPG

GPC = B // NCORES          # graphs per core = 32
NPC = GPC * NPG            # nodes per core = 2048
EPC = GPC * EPG            # edges per core = 16384
PAIRS = GPC // 2           # graph pairs per core = 16
CHUNK = 512                # edges per compute chunk
NCH = EPC // CHUNK         # chunks per core = 32
SEL_PG = 2                 # selected edges per graph
SELC = GPC * SEL_PG        # selected edges per core = 64

HDT = mybir.dt.float16
F32 = mybir.dt.float32
nph = np.float16

ALU = mybir.AluOpType
ACTF = mybir.ActivationFunctionType

_CACHE = {}


# ---------------------------------------------------------------- host pack

class Pack:
    """Accumulates weight blocks ([128, w] K-blocks) and bias blocks."""

    def __init__(self):
        self.w = {128: [], 256: [], 512: []}
        self.bias = []
        self.idx = {}       # key -> (width, start, nblocks)
        self.bidx = {}      # key -> (start, nblocks)
        self.layer_w = {}   # (layer, width) -> [start, count]

    def mark_layer(self, layer):
        self._layer = layer
        for w in (128, 256, 512):
            self.layer_w[(layer, w)] = [len(self.w[w]), 0]

    def add_w(self, key, W, width):
        W = np.asarray(W, np.float32)
        din, dout = W.shape
        assert dout <= width
        kb = -(-din // 128)
        Wp = np.zeros((kb * 128, width), np.float32)
        Wp[:din, :dout] = W
        start = len(self.w[width])
        for k in range(kb):
            self.w[width].append(Wp[k * 128:(k + 1) * 128])
        self.idx[key] = (width, start, kb)
        self.layer_w[(self._layer, width)][1] += kb

    def add_b(self, key, b):
        b = np.asarray(b, np.float32).reshape(-1)
        nb = -(-len(b) // 128)
        bp = np.zeros(nb * 128, np.float32)
        bp[:len(b)] = b
        start = len(self.bias)
        for k in range(nb):
            self.bias.append(bp[k * 128:(k + 1) * 128])
        self.bidx[key] = (start, nb)


def pack_weights(params):
    pk = Pack()
    for l in range(1, 7):
        pk.mark_layer(l)
        # edge MLP
        (W1, b1), (W2, b2), (W3, b3) = params[f'e{l}']
        nfx = NF if l == 1 else 128
        W1 = np.asarray(W1, np.float32)
        pk.add_w(f'e{l}r', W1[:nfx], 128)
        pk.add_w(f'e{l}c', W1[nfx:2 * nfx], 128)
        pk.add_w(f'e{l}e', W1[2 * nfx:], 128)
        pk.add_w(f'e{l}2', W2, 128)
        pk.add_w(f'e{l}3', W3, 512 if l < 6 else 128)
        pk.add_b(f'e{l}1', b1)
        pk.add_b(f'e{l}2', b2)
        pk.add_b(f'e{l}3', b3)
        if l == 6:
            break
        # node MLP a
        (Wa1, ba1), (Wa2, ba2), (Wa3, ba3) = params[f'n{l}a']
        Wa1 = np.asarray(Wa1, np.float32)
        pk.add_w(f'na{l}x', Wa1[:nfx], 256)
        pk.add_w(f'na{l}e', Wa1[nfx:], 256)
        pk.add_w(f'na{l}2', Wa2, 256)
        pk.add_w(f'na{l}3', Wa3, 256)
        pk.add_b(f'na{l}1', ba1)
        pk.add_b(f'na{l}2', ba2)
        # ba3 handled via replicated tile
        # node MLP b
        (Wb1, bb1), (Wb2, bb2) = params[f'n{l}b']
        Wb1 = np.asarray(Wb1, np.float32)
        pk.add_w(f'nb{l}x', Wb1[:nfx], 256)
        pk.add_w(f'nb{l}a', Wb1[nfx:], 256)
        pk.add_w(f'nb{l}2', Wb2, 128)
        pk.add_b(f'nb{l}1', bb1)
        pk.add_b(f'nb{l}2', bb2)
    pk.mark_layer(7)
    (Wh1, bh1), (Wh2, bh2) = params['head']
    pk.add_w('h1', Wh1, 128)
    pk.add_w('h2', Wh2, 128)
    pk.add_b('h1', bh1)
    pk.add_b('h2', bh2)

    ba3rep = np.zeros((128, 5, 256), np.float32)
    for l in range(1, 6):
        ba3rep[:, l - 1, :] = np.asarray(params[f'n{l}a'][2][1], np.float32)[None, :]
    return pk, ba3rep


# ---------------------------------------------------------------- host shard

def prep_inputs(inputs):
    """Shard + reorder + build one-hot matrices. Returns per-core input maps
    plus the pack object (weights are replicated)."""
    params = inputs['params']
    x = np.asarray(inputs['x'], np.float32)
    ea = np.asarray(inputs['edge_attr'], np.float32)
    ei = np.asarray(inputs['edge_index'], np.int64)
    batch = np.asarray(inputs['batch'], np.int64)
    sel_idx = np.asarray(inputs['sel_idx'], np.int64)
    eb = np.asarray(inputs['eb'], np.int64)

    row, col = ei[0], ei[1]
    gid = batch[row]
    assert np.array_equal(batch[col], gid), "edges must stay within one graph"
    # group edges by graph (stable) -> per-graph contiguous blocks of EPG
    perm = np.argsort(gid, kind='stable')
    counts = np.bincount(gid, minlength=B)
    assert np.all(counts == EPG), "expected uniform edges/graph"
    assert np.array_equal(batch, np.repeat(np.arange(B), NPG)), \
        "expected uniform nodes/graph"
    row_s, col_s, ea_s = row[perm], col[perm], ea[perm]
    inv = np.empty(E, np.int64)
    inv[perm] = np.arange(E)
    sel_pos = inv[sel_idx]            # position in reordered edge list
    # selected edges sorted by graph
    sel_g = eb
    assert np.all(np.bincount(sel_g, minlength=B) == SEL_PG)
    sord = np.argsort(sel_g, kind='stable')
    sel_pos = sel_pos[sord]
    sel_g = sel_g[sord]

    # in-degree over col (per node), computed per graph block
    deg = np.bincount(col_s, minlength=N).astype(np.float32)
    invdeg = 1.0 / np.maximum(deg, 1.0)

    pk, ba3rep = pack_weights(params)
    w128 = np.stack(pk.w[128]).astype(nph)
    w256 = np.stack(pk.w[256]).astype(nph)
    w512 = np.stack(pk.w[512]).astype(nph)
    biasT = np.stack(pk.bias, axis=1).astype(np.float32)     # [128, NB]
    ba3rep_h = ba3rep.reshape(128, 5 * 256)

    bn_g = np.asarray(params['bn_node'][0], np.float32)[None, :]
    bn_b = np.asarray(params['bn_node'][1], np.float32)[None, :]
    be_g = np.asarray(params['bn_edge'][0], np.float32)[None, :]
    be_b = np.asarray(params['bn_edge'][1], np.float32)[None, :]

    in_maps = []
    sel_copy_meta = None
    for cidx in range(NCORES):
        g0 = cidx * GPC
        n0, e0 = g0 * NPG, g0 * EPG
        xs = x[n0:n0 + NPC]                       # [2048, 32]
        eas = ea_s[e0:e0 + EPC]                   # [16384, 16]
        rows = row_s[e0:e0 + EPC] - n0            # local node idx 0..2047
        cols = col_s[e0:e0 + EPC] - n0

        # em layouts with partition dim = row-within-128-tile
        x_em = xs.reshape(NPC // 128, 128, NF).transpose(1, 0, 2).reshape(128, -1)
        e_em = eas.reshape(EPC // 128, 128, EF).transpose(1, 0, 2).reshape(128, -1)

        eidx = np.arange(EPC)
        pair = eidx // (2 * EPG)                  # pair of each edge
        rloc = rows - pair * 128                  # node idx within pair: 0..127
        cloc = cols - pair * 128
        assert rloc.min() >= 0 and rloc.max() < 128
        grow = np.zeros((128, EPC), np.float32)
        gcol = np.zeros((128, EPC), np.float32)
        grow[rloc, eidx] = 1.0
        gcol[cloc, eidx] = 1.0
        # scatter matrix, edge-major blocks: S_res[p, s*128+n]
        smean = np.zeros((128, EPC), np.float32)
        sub = eidx // 128                         # global subchunk
        pp = eidx % 128
        smean[pp, sub * 128 + cloc] = invdeg[col_s[e0:e0 + EPC]]

        # selection (layer-6) structures
        sp = sel_pos[g0 * SEL_PG:(g0 + GPC) * SEL_PG] - e0    # local positions
        sg = sel_g[g0 * SEL_PG:(g0 + GPC) * SEL_PG] - g0      # local graph ids
        spair = sp // (2 * EPG)
        srow = row_s[e0 + sp] - n0 - spair * 128
        scol = col_s[e0 + sp] - n0 - spair * 128
        sidx = np.arange(SELC)
        assert np.array_equal(spair, sidx // 4), \
            "expected 2 selected edges per graph in order"
        g6r = np.zeros((128, SELC), np.float32)
        g6c = np.zeros((128, SELC), np.float32)
        g6r[srow, sidx] = 1.0
        g6c[scol, sidx] = 1.0
        ssel = np.zeros((128, GPC), np.float32)
        ssel[sidx, sg] = 1.0

        # e5 extraction metadata: per chunk list of (src_off, dst, count)
        ch = sp // CHUNK
        off = sp % CHUNK
        meta = []
        for c in range(NCH):
            m_ = np.where(ch == c)[0]
            if len(m_) == 0:
                continue
            o = off[m_]
            runs = []
            s = 0
            while s < len(m_):
                t = s
                while t + 1 < len(m_) and o[t + 1] == o[t] + 1:
                    t += 1
                runs.append((int(o[s]), int(m_[s]), t - s + 1))
                s = t + 1
            meta.append((c, runs))
        if sel_copy_meta is None:
            sel_copy_meta = meta
        else:
            assert sel_copy_meta == meta, "sel layout must match across cores"

        in_maps.append({
            'x_em': x_em.astype(nph),
            'e0_em': e_em.astype(nph),
            'grow': grow.astype(nph),
            'gcol': gcol.astype(nph),
            'smean': smean.astype(nph),
            'g6r': g6r.astype(nph),
            'g6c': g6c.astype(nph),
            'ssel': ssel.astype(nph),
            'w128': w128, 'w256': w256, 'w512': w512,
            'bias': biasT, 'ba3rep': ba3rep_h,
            'bn_g': bn_g, 'bn_b': bn_b, 'be_g': be_g, 'be_b': be_b,
        })
    return in_maps, pk, sel_copy_meta


# ---------------------------------------------------------------- device

def build_program(pk, sel_meta):
    nc = bacc.Bacc("TRN2", target_bir_lowering=False, debug=False,
                   num_devices=NCORES)

    def din(name, shape, dtype):
        return nc.dram_tensor(name, shape, dtype, kind="ExternalInput")

    x_em_d = din('x_em', [128, (NPC // 128) * NF], HDT)
    e0_em_d = din('e0_em', [128, (EPC // 128) * EF], HDT)
    grow_d = din('grow', [128, EPC], HDT)
    gcol_d = din('gcol', [128, EPC], HDT)
    smean_d = din('smean', [128, EPC], HDT)
    g6r_d = din('g6r', [128, SELC], HDT)
    g6c_d = din('g6c', [128, SELC], HDT)
    ssel_d = din('ssel', [128, GPC], HDT)
    w128_d = din('w128', [len(pk.w[128]), 128, 128], HDT)
    w256_d = din('w256', [len(pk.w[256]), 128, 256], HDT)
    w512_d = din('w512', [len(pk.w[512]), 128, 512], HDT)
    bias_d = din('bias', [128, len(pk.bias)], F32)
    ba3rep_d = din('ba3rep', [128, 5 * 256], F32)
    bn_g_d = din('bn_g', [1, NF], F32)
    bn_b_d = din('bn_b', [1, NF], F32)
    be_g_d = din('be_g', [1, EF], F32)
    be_b_d = din('be_b', [1, EF], F32)
    out_d = nc.dram_tensor('out', [1, GPC], F32, kind="ExternalOutput")

    with tile.TileContext(nc) as tc:
        with (
            tc.tile_pool(name="const", bufs=1) as cpool,
            tc.tile_pool(name="wts", bufs=2) as wpool,
            tc.tile_pool(name="io", bufs=3) as iopool,
            tc.tile_pool(name="act", bufs=3) as apool,
            tc.tile_pool(name="pair", bufs=3) as ppool,
            tc.tile_pool(name="zp", bufs=1) as zpool,
            tc.tile_pool(name="xf", bufs=2) as xpool,
            tc.tile_pool(name="ps512", bufs=5, space="PSUM") as ps512,
            tc.tile_pool(name="psS", bufs=3, space="PSUM") as psS,
            tc.tile_pool(name="dram", bufs=1, space="DRAM") as dpool,
        ):
            build_body(nc, tc, pk, sel_meta, locals())
    nc.compile()
    return nc


def build_body(nc, tc, pk, sel_meta, env):
    cpool, wpool, iopool, apool = env['cpool'], env['wpool'], env['iopool'], env['apool']
    ppool, zpool, xpool = env['ppool'], env['zpool'], env['xpool']
    ps512, psS, dpool = env['ps512'], env['psS'], env['dpool']
    grow_d, gcol_d, smean_d = env['grow_d'], env['gcol_d'], env['smean_d']
    w128_d, w256_d, w512_d, bias_d = env['w128_d'], env['w256_d'], env['w512_d'], env['bias_d']

    # ---------------- constants
    ident = cpool.tile([128, 128], HDT, tag="ident")
    make_identity(nc, ident[:])
    grow = cpool.tile([128, EPC], HDT, tag="grow")
    nc.sync.dma_start(grow[:], grow_d[:])
    smean = cpool.tile([128, EPC], HDT, tag="smean")
    nc.sync.dma_start(smean[:], smean_d[:])
    biasT = cpool.tile([128, len(pk.bias)], F32, tag="bias")
    nc.sync.dma_start(biasT[:], bias_d[:])
    ba3rep = cpool.tile([128, 5, 256], F32, tag="ba3rep")
    nc.sync.dma_start(ba3rep[:], env['ba3rep_d'][:].rearrange("p (l w) -> p l w", l=5))
    x_em = cpool.tile([128, (NPC // 128) * NF], HDT, tag="x_em")
    nc.sync.dma_start(x_em[:], env['x_em_d'][:])
    e0_em = cpool.tile([128, (EPC // 128) * EF], HDT, tag="e0_em")
    nc.sync.dma_start(e0_em[:], env['e0_em_d'][:])

    def bias_ap(key, blk, parts=128):
        s, nb = pk.bidx[key]
        assert blk < nb
        return biasT[0:parts, s + blk:s + blk + 1]

    # ---------------- BatchNorm statistics (partial sums + AllReduce)
    ones_bf = cpool.tile([128, 1], HDT, tag="ones_bf")
    nc.gpsimd.memset(ones_bf[:], 1.0)
    ones_f = cpool.tile([128, 1], F32, tag="ones_f")
    nc.gpsimd.memset(ones_f[:], 1.0)

    x2 = cpool.tile([128, (NPC // 128) * NF], F32, tag="x2")
    nc.vector.tensor_tensor(out=x2[:], in0=x_em[:], in1=x_em[:], op=ALU.mult)

    ps_xs = psS.tile([1, NF], F32, tag="psS")
    ps_x2 = psS.tile([1, NF], F32, tag="psS")
    for t in range(NPC // 128):
        nc.tensor.matmul(ps_xs[:], lhsT=ones_bf[:], rhs=x_em[:, t * NF:(t + 1) * NF],
                         start=(t == 0), stop=(t == NPC // 128 - 1))
    for t in range(NPC // 128):
        nc.tensor.matmul(ps_x2[:], lhsT=ones_f[:], rhs=x2[:, t * NF:(t + 1) * NF],
                         start=(t == 0), stop=(t == NPC // 128 - 1))
    ps_es = psS.tile([1, EF], F32, tag="psS")
    ps_e2 = psS.tile([1, EF], F32, tag="psS")
    for t in range(EPC // 128):
        nc.tensor.matmul(ps_es[:], lhsT=ones_bf[:], rhs=e0_em[:, t * EF:(t + 1) * EF],
                         start=(t == 0), stop=(t == EPC // 128 - 1))
    ntile = EPC // 128            # 128 tiles of EF cols
    for g in range(4):            # square in 4 column groups to save SBUF
        e2 = iopool.tile([128, (ntile // 4) * EF], F32, tag="e2chunk")
        base = g * (ntile // 4)
        nc.vector.tensor_tensor(
            out=e2[:], in0=e0_em[:, base * EF:(base + ntile // 4) * EF],
            in1=e0_em[:, base * EF:(base + ntile // 4) * EF], op=ALU.mult)
        for t in range(ntile // 4):
            gt = base + t
            nc.tensor.matmul(ps_e2[:], lhsT=ones_f[:], rhs=e2[:, t * EF:(t + 1) * EF],
                             start=(gt == 0), stop=(gt == ntile - 1))

    stat = cpool.tile([1, 96], F32, tag="stat")
    nc.vector.tensor_copy(stat[:, 0:32], ps_xs[:])
    nc.vector.tensor_copy(stat[:, 32:64], ps_x2[:])
    nc.vector.tensor_copy(stat[:, 64:80], ps_es[:])
    nc.vector.tensor_copy(stat[:, 80:96], ps_e2[:])

    cc_in = dpool.tile([1, 96], F32)
    cc_out = dpool.tile([1, 96], F32)
    nc.sync.dma_start(cc_in[:], stat[:])
    nc.gpsimd.collective_compute(
        "AllReduce", ALU.add, replica_groups=[list(range(NCORES))],
        ins=[cc_in.opt()], outs=[cc_out.opt()])
    statg = cpool.tile([1, 96], F32, tag="statg")
    nc.sync.dma_start(statg[:], cc_out[:])

    # affine params in free-dim layout, then bounce to partition layout
    aff = cpool.tile([1, 96], F32, tag="aff")
    tmp = cpool.tile([1, 512], F32, tag="bntmp")
    gparams = [env['bn_g_d'], env['bn_b_d'], env['be_g_d'], env['be_b_d']]
    gtiles = []
    for i, d in enumerate(gparams):
        t = cpool.tile([1, [NF, NF, EF, EF][i]], F32, tag=f"bnp{i}")
        nc.sync.dma_start(t[:], d[:])
        gtiles.append(t)
    for (s0, g_t, b_t, cnt, ntot, t0) in (
            (0, gtiles[0], gtiles[1], NF, float(N), 0),
            (64, gtiles[2], gtiles[3], EF, float(E), 256)):
        def seg(i):
            return tmp[:, t0 + i * cnt:t0 + (i + 1) * cnt]
        mean, ex2, m2, varr, std, rstd, ms, veps = (seg(i) for i in range(8))
        nc.vector.tensor_scalar_mul(mean, statg[:, s0:s0 + cnt], 1.0 / ntot)
        nc.vector.tensor_scalar_mul(ex2, statg[:, s0 + cnt:s0 + 2 * cnt], 1.0 / ntot)
        nc.vector.tensor_tensor(out=m2, in0=mean, in1=mean, op=ALU.mult)
        nc.vector.tensor_tensor(out=varr, in0=ex2, in1=m2, op=ALU.subtract)
        nc.vector.tensor_scalar_add(veps, varr, 1e-5)
        nc.scalar.activation(std, veps, ACTF.Sqrt)
        nc.vector.reciprocal(rstd, std)
        scale = aff[:, s0:s0 + cnt]
        nc.vector.tensor_tensor(out=scale, in0=g_t[:], in1=rstd, op=ALU.mult)
        shift = aff[:, s0 + cnt:s0 + 2 * cnt]
        nc.vector.tensor_tensor(out=ms, in0=mean, in1=scale, op=ALU.mult)
        nc.vector.tensor_tensor(out=shift, in0=b_t[:], in1=ms, op=ALU.subtract)
    cc_aff = dpool.tile([1, 96], F32)
    nc.sync.dma_start(cc_aff[:], aff[:])
    affp = []
    for i, (s0, cnt) in enumerate(((0, NF), (NF, NF), (64, EF), (64 + EF, EF))):
        t = cpool.tile([cnt, 1], F32, tag=f"afft{i}")
        nc.sync.dma_start(t[:], cc_aff[:, s0:s0 + cnt].rearrange("a b -> b a"))
        affp.append(t[:])
    sc_x, sh_x, sc_e, sh_e = affp

    # ---------------- x0_fm: transpose + BN affine, zero-padded to 128 rows
    x0_fm = cpool.tile([128, NPC], HDT, tag="x0fm")
    nc.gpsimd.memset(x0_fm[:], 0.0)
    for t in range(NPC // 128):
        pst = psS.tile([NF, 128], HDT, tag="psS")
        nc.tensor.transpose(pst[:], x_em[:, t * NF:(t + 1) * NF], ident[:])
        nc.vector.tensor_scalar(
            out=x0_fm[0:NF, t * 128:(t + 1) * 128], in0=pst[:],
            scalar1=sc_x, scalar2=sh_x, op0=ALU.mult, op1=ALU.add)

    # ---------------- edge DRAM ping-pong buffers
    eA = dpool.tile([NCH, 128, 4 * CHUNK], HDT)
    eB = dpool.tile([NCH, 128, 4 * CHUNK], HDT)

    def wslice(wtile, lstart, key, blk, width):
        w, s, nb = pk.idx[key]
        assert w == width and blk < nb
        return wtile[:, s - lstart + blk]

    e5sel = cpool.tile([128, 4, SELC], HDT, tag="e5sel")
    xfm_cur = x0_fm

    # ================= layers 1..5
    for l in range(1, 6):
        nfx = NF if l == 1 else 128
        s128, n128 = pk.layer_w[(l, 128)]
        s256, n256 = pk.layer_w[(l, 256)]
        s512, n512 = pk.layer_w[(l, 512)]
        wl128 = wpool.tile([128, n128, 128], HDT, tag="w128")
        nc.sync.dma_start(wl128[:], w128_d[s128:s128 + n128].rearrange("n p w -> p n w"))
        wl256 = wpool.tile([128, n256, 256], HDT, tag="w256")
        nc.sync.dma_start(wl256[:], w256_d[s256:s256 + n256].rearrange("n p w -> p n w"))
        wl512 = wpool.tile([128, n512, 512], HDT, tag="w512")
        nc.sync.dma_start(wl512[:], w512_d[s512:s512 + n512].rearrange("n p w -> p n w"))

        W = lambda key, blk=0, width=128: wslice(
            {128: wl128, 256: wl256, 512: wl512}[width],
            {128: s128, 256: s256, 512: s512}[width], key, blk, width)

        # ---- z projections (node-major), per pair
        z_row = zpool.tile([128, PAIRS * 128], HDT, tag="zrow")
        z_col = zpool.tile([128, PAIRS * 128], HDT, tag="zcol")
        z_a = zpool.tile([128, PAIRS * 256], HDT, tag="za")
        for p in range(PAIRS):
            xs = xfm_cur[:, p * 128:(p + 1) * 128]
            pz = psS.tile([128, 128], F32, tag="psS")
            nc.tensor.matmul(pz[:], lhsT=xs, rhs=W(f'e{l}r'), start=True, stop=True)
            nc.vector.tensor_copy(z_row[:, p * 128:(p + 1) * 128], pz[:])
            pz2 = psS.tile([128, 128], F32, tag="psS")
            nc.tensor.matmul(pz2[:], lhsT=xs, rhs=W(f'e{l}c'), start=True, stop=True)
            nc.vector.tensor_copy(z_col[:, p * 128:(p + 1) * 128], pz2[:])
            pz3 = psS.tile([128, 256], F32, tag="psS")
            nc.tensor.matmul(pz3[:], lhsT=xs, rhs=W(f'na{l}x', 0, 256), start=True, stop=True)
            nc.vector.tensor_copy(z_a[:, p * 256:(p + 1) * 256], pz3[:])

        xfm_next = xpool.tile([128, NPC], HDT, tag="xfm")
        m_tiles, agg_tiles = {}, {}
        sel_meta_d = dict(sel_meta) if l == 5 else {}

        def edge_na_stage(c):
            p = c // 2
            # ---- edge features for this chunk (feature-major)
            if l == 1:
                e_prev = iopool.tile([128, CHUNK], HDT, tag="e0fm")
                nc.gpsimd.memset(e_prev[:], 0.0)
                for i in range(4):
                    u = c * 4 + i
                    pst = psS.tile([EF, 128], HDT, tag="psS")
                    nc.tensor.transpose(pst[:], e0_em[:, u * EF:(u + 1) * EF], ident[:])
                    nc.vector.tensor_scalar(
                        out=e_prev[0:EF, i * 128:(i + 1) * 128], in0=pst[:],
                        scalar1=sc_e, scalar2=sh_e, op0=ALU.mult, op1=ALU.add)
                ek = 1
                eblk = lambda k: e_prev[:]
            else:
                e_prev = iopool.tile([128, 4 * CHUNK], HDT, tag="eprev")
                src = eA if l % 2 == 0 else eB
                nc.sync.dma_start(e_prev[:], src[c])
                ek = 4
                eblk = lambda k: e_prev[:, k * CHUNK:(k + 1) * CHUNK]

            gr = grow[:, c * CHUNK:(c + 1) * CHUNK]
            gc_t = iopool.tile([128, CHUNK], HDT, tag="gcol")
            nc.sync.dma_start(gc_t[:], gcol_d[:, c * CHUNK:(c + 1) * CHUNK])

            # ---- edge MLP layer 1
            ph1 = ps512.tile([128, CHUNK], F32, tag="ps512")
            nc.tensor.matmul(ph1[:], lhsT=z_row[:, p * 128:(p + 1) * 128], rhs=gr,
                             start=True, stop=False)
            nc.tensor.matmul(ph1[:], lhsT=z_col[:, p * 128:(p + 1) * 128], rhs=gc_t[:],
                             start=False, stop=False)
            for k in range(ek):
                nc.tensor.matmul(ph1[:], lhsT=W(f'e{l}e', k), rhs=eblk(k),
                                 start=False, stop=(k == ek - 1))
            h1 = apool.tile([128, CHUNK], HDT, tag="h1")
            nc.scalar.activation(h1[:], ph1[:], ACTF.Relu, bias=bias_ap(f'e{l}1', 0))
            # ---- edge MLP layer 2
            ph2 = ps512.tile([128, CHUNK], F32, tag="ps512")
            nc.tensor.matmul(ph2[:], lhsT=W(f'e{l}2'), rhs=h1[:], start=True, stop=True)
            h2 = apool.tile([128, CHUNK], HDT, tag="h2")
            nc.scalar.activation(h2[:], ph2[:], ACTF.Relu, bias=bias_ap(f'e{l}2', 0))
            # ---- edge MLP layer 3 -> e_new (512 wide)
            e_new = iopool.tile([128, 4 * CHUNK], HDT, tag="enew")
            for k in range(4):
                pe = ps512.tile([128, CHUNK], F32, tag="ps512")
                nc.tensor.matmul(pe[:], lhsT=W(f'e{l}3', 0, 512)[:, k * 128:(k + 1) * 128],
                                 rhs=h2[:], start=True, stop=True)
                nc.vector.tensor_scalar_add(
                    e_new[:, k * CHUNK:(k + 1) * CHUNK], pe[:], bias_ap(f'e{l}3', k))
            if l < 5:
                dst = eB if l % 2 == 0 else eA
                if l == 1:
                    dst = eA
                nc.sync.dma_start(dst[c], e_new[:])
            if l == 5 and c in sel_meta_d:
                for (o, d0, cnt) in sel_meta_d[c]:
                    for k in range(4):
                        nc.vector.tensor_copy(
                            e5sel[:, k, d0:d0 + cnt],
                            e_new[:, k * CHUNK + o:k * CHUNK + o + cnt])

            # ---- node MLP a (layers 1-3), m in edge-major
            a1 = apool.tile([128, 2, CHUNK], HDT, tag="a1")
            for j in range(2):
                pa = ps512.tile([128, CHUNK], F32, tag="ps512")
                nc.tensor.matmul(pa[:], lhsT=z_a[:, p * 256 + j * 128:p * 256 + (j + 1) * 128],
                                 rhs=gr, start=True, stop=False)
                for k in range(4):
                    nc.tensor.matmul(pa[:], lhsT=W(f'na{l}e', k, 256)[:, j * 128:(j + 1) * 128],
                                     rhs=e_new[:, k * CHUNK:(k + 1) * CHUNK],
                                     start=False, stop=(k == 3))
                nc.scalar.activation(a1[:, j, :], pa[:], ACTF.Relu,
                                     bias=bias_ap(f'na{l}1', j))
            a2 = apool.tile([128, 2, CHUNK], HDT, tag="a2")
            for j in range(2):
                pa = ps512.tile([128, CHUNK], F32, tag="ps512")
                for k in range(2):
                    nc.tensor.matmul(pa[:], lhsT=W(f'na{l}2', k, 256)[:, j * 128:(j + 1) * 128],
                                     rhs=a1[:, k, :], start=(k == 0), stop=(k == 1))
                nc.scalar.activation(a2[:, j, :], pa[:], ACTF.Relu,
                                     bias=bias_ap(f'na{l}2', j))
            if c % 2 == 0:
                m_tiles[p] = ppool.tile([128, 8, 256], HDT, tag="mem",
                                        name=f"mem_l{l}_p{p}")
            m_em = m_tiles[p]
            for q in range(4):
                pm = psS.tile([128, 256], F32, tag="psS")
                for k in range(2):
                    nc.tensor.matmul(pm[:], lhsT=a2[:, k, q * 128:(q + 1) * 128],
                                     rhs=W(f'na{l}3', k, 256), start=(k == 0), stop=(k == 1))
                nc.vector.tensor_tensor(out=m_em[:, (c % 2) * 4 + q, :], in0=pm[:],
                                        in1=ba3rep[:, l - 1, :], op=ALU.add)

        def scatter_stage(p):
            # ---- scatter-mean (per pair), feature-major agg
            m_em = m_tiles.pop(p)
            agg = ppool.tile([128, 2, 128], HDT, tag="agg")
            agg_tiles[p] = agg
            for j in range(2):
                pg = psS.tile([128, 128], F32, tag="psS")
                for q in range(8):
                    s = p * 8 + q
                    nc.tensor.matmul(pg[:], lhsT=m_em[:, q, j * 128:(j + 1) * 128],
                                     rhs=smean[:, s * 128:(s + 1) * 128],
                                     start=(q == 0), stop=(q == 7))
                nc.vector.tensor_copy(agg[:, j, :], pg[:])

        def nb_stage(p):
            # ---- node MLP b
            agg = agg_tiles.pop(p)
            b1 = ppool.tile([128, 2, 128], HDT, tag="b1")
            for j in range(2):
                pb = psS.tile([128, 128], F32, tag="psS")
                nc.tensor.matmul(pb[:], lhsT=W(f'nb{l}x', 0, 256)[:, j * 128:(j + 1) * 128],
                                 rhs=xfm_cur[:, p * 128:(p + 1) * 128],
                                 start=True, stop=False)
                for k in range(2):
                    nc.tensor.matmul(pb[:], lhsT=W(f'nb{l}a', k, 256)[:, j * 128:(j + 1) * 128],
                                     rhs=agg[:, k, :], start=False, stop=(k == 1))
                nc.scalar.activation(b1[:, j, :], pb[:], ACTF.Relu,
                                     bias=bias_ap(f'nb{l}1', j))
            px = psS.tile([128, 128], F32, tag="psS")
            for k in range(2):
                nc.tensor.matmul(px[:], lhsT=W(f'nb{l}2', k), rhs=b1[:, k, :],
                                 start=(k == 0), stop=(k == 1))
            nc.vector.tensor_scalar_add(
                xfm_next[:, p * 128:(p + 1) * 128], px[:], bias_ap(f'nb{l}2', 0))

        # software pipeline: delay each pair's scatter by one chunk and its
        # node-MLP by two, so PE always has independent matmuls queued while
        # the epilogue chains drain.
        for c in range(NCH):
            edge_na_stage(c)
            if c >= 2 and c % 2 == 0:
                scatter_stage((c - 2) // 2)
            if c >= 3 and c % 2 == 1:
                nb_stage((c - 3) // 2)
        scatter_stage(PAIRS - 1)
        nb_stage(PAIRS - 1)
        xfm_cur = xfm_next

    # ================= layer 6 (selected edges only) + head
    l = 6
    s128, n128 = pk.layer_w[(6, 128)]
    sh128, nh128 = pk.layer_w[(7, 128)]
    wl128 = wpool.tile([128, n128 + nh128, 128], HDT, tag="w128")
    nc.sync.dma_start(wl128[:], w128_d[s128:s128 + n128 + nh128].rearrange("n p w -> p n w"))
    W6 = lambda key, blk=0: wslice(wl128, s128, key, blk, 128)

    g6r = cpool.tile([128, SELC], HDT, tag="g6r")
    nc.sync.dma_start(g6r[:], env['g6r_d'][:])
    g6c = cpool.tile([128, SELC], HDT, tag="g6c")
    nc.sync.dma_start(g6c[:], env['g6c_d'][:])
    ssel = cpool.tile([128, GPC], HDT, tag="ssel")
    nc.sync.dma_start(ssel[:], env['ssel_d'][:])

    z6r = zpool.tile([128, PAIRS * 128], HDT, tag="zrow")
    z6c = zpool.tile([128, PAIRS * 128], HDT, tag="zcol")
    for p in range(PAIRS):
        xs = xfm_cur[:, p * 128:(p + 1) * 128]
        pz = psS.tile([128, 128], F32, tag="psS")
        nc.tensor.matmul(pz[:], lhsT=xs, rhs=W6('e6r'), start=True, stop=True)
        nc.vector.tensor_copy(z6r[:, p * 128:(p + 1) * 128], pz[:])
        pz2 = psS.tile([128, 128], F32, tag="psS")
        nc.tensor.matmul(pz2[:], lhsT=xs, rhs=W6('e6c'), start=True, stop=True)
        nc.vector.tensor_copy(z6c[:, p * 128:(p + 1) * 128], pz2[:])

    pg6 = psS.tile([128, SELC], F32, tag="psS")
    for p in range(PAIRS):
        sl = slice(4 * p, 4 * p + 4)
        nc.tensor.matmul(pg6[:, sl], lhsT=z6r[:, p * 128:(p + 1) * 128],
                         rhs=g6r[:, sl], start=True, stop=False)
        nc.tensor.matmul(pg6[:, sl], lhsT=z6c[:, p * 128:(p + 1) * 128],
                         rhs=g6c[:, sl], start=False, stop=True)
    ph6 = psS.tile([128, SELC], F32, tag="psS")
    for k in range(4):
        nc.tensor.matmul(ph6[:], lhsT=W6('e6e', k), rhs=e5sel[:, k, :],
                         start=(k == 0), stop=(k == 3))
    ph6s = cpool.tile([128, SELC], F32, tag="ph6s")
    nc.vector.tensor_copy(ph6s[:], ph6[:])
    h6a = cpool.tile([128, SELC], HDT, tag="h6a")
    nc.vector.tensor_tensor(out=h6a[:], in0=pg6[:], in1=ph6s[:], op=ALU.add)
    h6 = cpool.tile([128, SELC], HDT, tag="h6")
    nc.scalar.activation(h6[:], h6a[:], ACTF.Relu, bias=bias_ap('e61', 0))
    ph7 = psS.tile([128, SELC], F32, tag="psS")
    nc.tensor.matmul(ph7[:], lhsT=W6('e62'), rhs=h6[:], start=True, stop=True)
    h7 = cpool.tile([128, SELC], HDT, tag="h7")
    nc.scalar.activation(h7[:], ph7[:], ACTF.Relu, bias=bias_ap('e62', 0))
    pe6 = psS.tile([128, SELC], F32, tag="psS")
    nc.tensor.matmul(pe6[:], lhsT=W6('e63'), rhs=h7[:], start=True, stop=True)
    e6f = cpool.tile([128, SELC], HDT, tag="e6f")
    nc.vector.tensor_scalar_add(e6f[:], pe6[:], bias_ap('e63', 0))

    # transpose to edge-major, pad, project per graph
    e6em = cpool.tile([128, 128], HDT, tag="e6em")
    nc.gpsimd.memset(e6em[:], 0.0)
    pt = psS.tile([SELC, 128], HDT, tag="psS")
    nc.tensor.transpose(pt[:], e6f[:], ident[:])
    nc.vector.tensor_copy(e6em[0:SELC, :], pt[:])
    py = psS.tile([128, GPC], F32, tag="psS")
    nc.tensor.matmul(py[:], lhsT=e6em[:], rhs=ssel[:], start=True, stop=True)
    ysb = cpool.tile([128, GPC], HDT, tag="ysb")
    nc.vector.tensor_copy(ysb[:], py[:])
    phh = psS.tile([128, GPC], F32, tag="psS")
    nc.tensor.matmul(phh[:], lhsT=W6('h1'), rhs=ysb[:], start=True, stop=True)
    hh = cpool.tile([128, GPC], HDT, tag="hh")
    nc.scalar.activation(hh[:], phh[:], ACTF.Relu, bias=bias_ap('h1', 0))
    po = psS.tile([1, GPC], F32, tag="psS")
    nc.tensor.matmul(po[:], lhsT=W6('h2')[:, 0:1], rhs=hh[:], start=True, stop=True)
    osb = cpool.tile([1, GPC], F32, tag="osb")
    nc.vector.tensor_scalar_add(osb[:], po[:], bias_ap('h2', 0, parts=1))
    nc.sync.dma_start(env['out_d'][:], osb[:])


def kernel(**inputs) -> np.ndarray:
    in_maps, pk, sel_meta = prep_inputs(inputs)
    key = 'prog'
    if key not in _CACHE:
        _CACHE[key] = build_program(pk, sel_meta)
    nc = _CACHE[key]
    res = run_bass_kernel_spmd(nc, in_maps, list(range(NCORES)))
    kernel.last_results = res
    out = np.concatenate([res.results[c]['out'].reshape(GPC) for c in range(NCORES)])
    return out.astype(np.float32).reshape(B, 1)
